# revision 17
# baseline (speedup 1.0000x reference)
# Trainium2 Bass kernel for nn_DirectRanker (ragged_sequence).
#
# Math shortcut: result = tanh((sorted_enc[:,1:,:] - sorted_enc[:,:1,:]) @ W.T)
# commutes with the linear map, so per-row scores s = encodes @ W.T are
# computed FIRST (the memory-bound part: 512 MiB of bf16 streamed once), and
# the per-group sort/diff/tanh runs on the tiny [N] score vector:
#   result[g, k-1] = tanh(s_sorted[g, k] - s_sorted[g, 0]),  k = 1..63
#
# Sharding: groups split across 8 cores (2048 groups/core), no cross-core
# communication.
#
# v4: encodes is pre-arranged ON HOST to [2, 8, 128, 2, 8192] bf16
# (d-chunk, tile-pair, d, tile, col) so that every encode DMA is ONE dense
# 4 MiB linear HBM extent ([128, 16384] with partition stride 32 KiB) --
# linear extents measure 383 GB/s/core vs 308 for the strided layout.
# TensorE computes the matvec in its native orientation:
#   psum[1, 512] += W_chunk[128, 1].T @ ET_chunk[128, 512]
# (no on-chip transposes).  PSUM exits to SBUF fp16 split 3:1 between
# ScalarE and VectorE.  A single SBUF->SBUF DMA per 128-group tile
# relayouts the flat scores into [group(partition), elem(free)] for the
# sort.  Big streams ride the sync HWDGE ring; dependent small DMAs (score
# relayout, result store) ride the gpsimd SWDGE ring so they can't
# head-of-line block the encode stream.
#
# The sort chain for tile T is EMITTED one tile late (after tile T+1's
# matvec/exits).  Engine queues are strict FIFO: emitted in natural order,
# tile T's sort ops (which wait on the T score relayout) would sit in the
# Act/DVE/GpSimd queues AHEAD of tile T+1's psum exits and stall the whole
# psum -> matmul -> encode-DMA pipeline (measured: 8.5 us matmul stalls per
# pair, DMA duty 75%).  With the one-tile lag every emitted op's deps are
# already in flight, so the exits stream back-to-back.
#
# Exact stable argsort over y within each 64-row group: integer keys
#   key = (y * 2^23 + 2^23) * 64 | elem_index     (y is a multiple of 2^-23)
# are sorted through their f32 bitcast views (monotone for positive int32;
# keys lie in [2^29, 2^30) so the views are normal floats) with 8 rounds of
# DVE max8 + match_replace; perm = low 6 bits of the sorted keys. The score
# permutation runs on gpsimd local_scatter (fp16 as int16), ranks coming
# from scattering a descending iota by perm.
import os
from contextlib import ExitStack

import ml_dtypes
import numpy as np

import concourse.bacc as bacc
import concourse.mybir as mybir
import concourse.tile as tile
from concourse.bass_utils import run_bass_kernel_spmd

N_CORES = 8
N = 1048576
D = 256
G = 64
NG = N // G                # 16384 groups
ROWS = N // N_CORES        # 131072 rows per core
GPC = NG // N_CORES        # 2048 groups per core
T_TILES = GPC // 128       # 16 tiles of 128 groups (8192 rows) per core
PAIRS = T_TILES // 2       # 8 DMA pairs (2 tiles per 4 MiB extent)
RPT = 128 * G              # rows per tile = 8192
MM_N = 512                 # moving free size per matmul (1 psum bank)
EXIT_N = 2048              # scores per psum exit copy (4 psum banks)
F32 = mybir.dt.float32
F16 = mybir.dt.float16
BF16 = mybir.dt.bfloat16
I32 = mybir.dt.int32
I16 = mybir.dt.int16
Alu = mybir.AluOpType
Act = mybir.ActivationFunctionType

_built = {}


def _build_nc():
    nc = bacc.Bacc("TRN2", target_bir_lowering=False, debug=False,
                   num_devices=N_CORES)
    # host-prearranged encodes: [d-chunk, pair, d, tile-in-pair, col]
    et_in = nc.dram_tensor("et", [2, PAIRS, 128, 2, RPT], BF16,
                           kind="ExternalInput")
    y_in = nc.dram_tensor("y_coord", [ROWS], F32, kind="ExternalInput")
    w_in = nc.dram_tensor("w", [1, D], F32, kind="ExternalInput")
    out = nc.dram_tensor("result", [GPC * (G - 1)], F32, kind="ExternalOutput")

    out_r = out.ap().rearrange("(t p k) -> t p k", p=128, k=G - 1)

    with tile.TileContext(nc) as tc, ExitStack() as ctx:
        const_pool = ctx.enter_context(tc.tile_pool(name="const", bufs=1))
        epool = ctx.enter_context(tc.tile_pool(name="e", bufs=2))
        sfpool = ctx.enter_context(tc.tile_pool(name="sf", bufs=3))
        spool = ctx.enter_context(tc.tile_pool(name="s", bufs=6))
        scr_pool = ctx.enter_context(tc.tile_pool(name="scr", bufs=3))
        ps_pool = ctx.enter_context(
            tc.tile_pool(name="ps", bufs=2, space="PSUM"))

        # W FIRST on the (otherwise idle) scalar HWDGE ring so it lands in
        # ~2 us -- on the sync ring it would queue behind 8 MiB of encodes
        # and the first LDWEIGHTS would stall ~50 us.
        wsb = const_pool.tile([128, 2], F32)
        nc.scalar.dma_start(wsb[:],
                            w_in.ap()[0, :].rearrange("(c p) -> p c", p=128))
        wsb_h = const_pool.tile([128, 2], BF16)
        nc.vector.tensor_copy(wsb_h[:], wsb[:])

        etc_tiles = {}

        def load_pair(pair):
            etc0 = epool.tile([128, 2 * RPT], BF16, tag="etc0")
            nc.sync.dma_start(
                etc0[:], et_in.ap()[0, pair].rearrange("p t n -> p (t n)"))
            etc1 = epool.tile([128, 2 * RPT], BF16, tag="etc1")
            nc.sync.dma_start(
                etc1[:], et_in.ap()[1, pair].rearrange("p t n -> p (t n)"))
            etc_tiles[pair] = (etc0, etc1)

        load_pair(0)
        # all of y for this core, loaded up-front on the gpsimd SWDGE ring
        # (so its 2048 small descriptors can't head-of-line block the encode
        # stream on the sync ring): y_all[p, T, u] = y[(T*128+p)*64 + u]
        y_all = const_pool.tile([128, T_TILES, G], F32)
        nc.gpsimd.dma_start(
            y_all[:], y_in.ap().rearrange("(t p u) -> p t u", p=128, u=G))
        # free-dim iota (elem index within group) for the sort keys
        iota_i = const_pool.tile([128, G], I32)
        nc.gpsimd.iota(iota_i[:], pattern=[[1, G]], base=0, channel_multiplier=0)
        # descending iota (63..0) as int16: data for the rank-producing scatter
        iota_d16 = const_pool.tile([128, G], I16)
        nc.gpsimd.iota(iota_d16[:], pattern=[[-1, G]], base=G - 1,
                       channel_multiplier=0)

        s_tiles = {}
        rank_tiles = {}
        ss_tiles = {}
        th_tiles = {}

        def rank_chain(T):
            """Keys + sort + rank for tile T.  Depends ONLY on y_all (loaded
            at kernel start), so these DVE/gpsimd ops never wait and can be
            emitted arbitrarily early."""
            # keys on DVE: k64 = int(y*2^29 + 2^29) (exact in fp32: y is a
            # multiple of 2^-23, so y*2^29 = 64k and 2^29 + 64k have <= 24
            # significant bits over a 2^6 ulp); keys = k64 | elem_index
            k64 = spool.tile([128, G], I32, tag="k64")
            nc.vector.tensor_scalar(out=k64[:], in0=y_all[:, T, :],
                                    scalar1=float(1 << 29),
                                    scalar2=float(1 << 29),
                                    op0=Alu.mult, op1=Alu.add)
            keys = spool.tile([128, G], I32, tag="keys")
            nc.vector.tensor_tensor(out=keys[:], in0=k64[:], in1=iota_i[:],
                                    op=Alu.bitwise_or)

            # full descending sort of the int keys on DVE (f32 bitcast views)
            sorted_i = spool.tile([128, G], I32, tag="sorted")
            wka = scr_pool.tile([128, G], I32, tag="wka")
            wkb = scr_pool.tile([128, G], I32, tag="wkb")
            src = keys
            dst_t = wka
            for r in range(8):
                nc.vector.max(sorted_i[:, r * 8:(r + 1) * 8].bitcast(F32),
                              src[:].bitcast(F32))
                if r < 7:
                    nc.vector.match_replace(
                        dst_t[:].bitcast(F32),
                        sorted_i[:, r * 8:(r + 1) * 8].bitcast(F32),
                        src[:].bitcast(F32), 0.0)
                    src, dst_t = dst_t, (wkb if dst_t is wka else wka)

            # perm (descending argsort) = low 6 bits; int16 for the scatter
            perm32 = scr_pool.tile([128, G], I32, tag="perm32")
            nc.vector.tensor_scalar(out=perm32[:], in0=sorted_i[:],
                                    scalar1=63, scalar2=None,
                                    op0=Alu.bitwise_and)
            perm16 = spool.tile([128, G], I16, tag="perm16")
            nc.vector.tensor_copy(perm16[:], perm32[:])
            # rank_asc[i] = position of element i in ascending order
            rank16 = spool.tile([128, G], I16, tag="rank16")
            nc.gpsimd.local_scatter(rank16[:], iota_d16[:], perm16[:],
                                    channels=128, num_elems=G, num_idxs=G)
            rank_tiles[T] = rank16

        def score_scatter(T):
            """Permute tile T's fp16 scores by rank in one gpsimd scatter
            (rank16 and s_t both long since materialized)."""
            s_t = s_tiles.pop(T)
            rank16 = rank_tiles.pop(T)
            ssort = spool.tile([128, G], I16, tag="ssort")
            nc.gpsimd.local_scatter(ssort[:], s_t[:].bitcast(I16), rank16[:],
                                    channels=128, num_elems=G, num_idxs=G)
            ss_tiles[T] = ssort

        def tanh_store(T):
            """negs0 + tanh on Act (ssort long done -> no Act stall)."""
            ssort = ss_tiles.pop(T)
            ssf = ssort[:].bitcast(F16)
            negs0 = spool.tile([128, 1], F32, tag="negs0")
            nc.scalar.mul(negs0[:], ssf[:, 0:1], -1.0)
            th = spool.tile([128, G - 1], F32, tag="th")
            nc.scalar.activation(th[:], ssf[:, 1:G], Act.Tanh,
                                 bias=negs0[:], scale=1.0)
            th_tiles[T] = th

        def store_out(T):
            nc.gpsimd.dma_start(out_r[T], th_tiles.pop(T))

        # rank chains for the first two tiles run during the DMA head while
        # DVE/gpsimd are otherwise idle
        rank_chain(0)
        rank_chain(1)

        for pair in range(PAIRS):
            if pair > 0:
                load_pair(pair)
            etc0, etc1 = etc_tiles.pop(pair)

            for t in range(2):
                T = pair * 2 + t
                # ---- scores for tile T (native-orientation PE matvec) ----
                sflat = sfpool.tile([1, RPT], F16, tag="sflat")
                for j in range(RPT // EXIT_N):
                    ps = ps_pool.tile([1, EXIT_N], F32, tag="ps")
                    for q in range(EXIT_N // MM_N):
                        c0 = t * RPT + j * EXIT_N + q * MM_N
                        nc.tensor.matmul(ps[:, q * MM_N:(q + 1) * MM_N],
                                         wsb_h[:, 0:1], etc0[:, c0:c0 + MM_N],
                                         start=True, stop=False)
                        nc.tensor.matmul(ps[:, q * MM_N:(q + 1) * MM_N],
                                         wsb_h[:, 1:2], etc1[:, c0:c0 + MM_N],
                                         start=False, stop=True)
                    # exit psum -> sbuf fp16, 3:1 Act : DVE
                    dst = sflat[:, j * EXIT_N:(j + 1) * EXIT_N]
                    if j == 1:
                        nc.vector.tensor_copy(dst, ps[:])
                    else:
                        nc.scalar.copy(dst, ps[:])

                # ---- relayout flat scores -> [group(partition), elem] ----
                # issued on the Act HWDGE ring: its dep (the exits) is Act's
                # own just-completed work, so the issue can't block anything;
                # on the gpsimd ring it would head-of-line block the
                # scatters behind it for a whole tile.
                s_t = spool.tile([128, G], F16, tag="s")
                nc.scalar.dma_start(s_t[:], sflat[:])
                s_tiles[T] = s_t

                # lagged, decoupled tail stages: every op emitted below has
                # its deps already complete (or completing this tile)
                if T + 2 < T_TILES:
                    rank_chain(T + 2)
                if T >= 1:
                    score_scatter(T - 1)
                if T >= 2:
                    tanh_store(T - 2)
                if T >= 3:
                    store_out(T - 3)
        score_scatter(T_TILES - 1)
        tanh_store(T_TILES - 2)
        tanh_store(T_TILES - 1)
        for T in (T_TILES - 3, T_TILES - 2, T_TILES - 1):
            store_out(T)

    nc.compile()
    return nc


last_results = None


def kernel(encodes, y_coord, W, x_coord=None):
    global last_results
    if "nc" not in _built:
        _built["nc"] = _build_nc()
    nc = _built["nc"]

    e16 = np.asarray(encodes).astype(ml_dtypes.bfloat16)
    y_coord = np.ascontiguousarray(np.asarray(y_coord, dtype=np.float32))
    W = np.ascontiguousarray(np.asarray(W, dtype=np.float32))

    in_maps = []
    for c in range(N_CORES):
        # [ROWS, 256] -> [256, ROWS] -> [2, 128, 8, 2, 8192] (c d pair t n)
        # -> [2, 8, 128, 2, 8192] (c pair d t n), each [pair] slice one
        # dense 4 MiB extent per d-chunk
        et_c = np.ascontiguousarray(
            e16[c * ROWS:(c + 1) * ROWS].T
            .reshape(2, 128, PAIRS, 2, RPT)
            .transpose(0, 2, 1, 3, 4))
        in_maps.append({
            "et": et_c,
            "y_coord": y_coord[c * ROWS:(c + 1) * ROWS],
            "w": W,
        })
    # Only request tracing when the axon NTFF hook is importable; otherwise
    # force it off (bass_utils would crash importing antenv.axon_hooks if
    # BASS_TRACE leaked into the environment without the shim installed).
    want_trace = bool(os.environ.get("BASS_TRACE"))
    if want_trace:
        try:
            from antenv.axon_hooks import get_axon_ntff_profile_hook  # noqa: F401
        except ImportError:
            want_trace = False
            os.environ["BASS_NEVER_TRACE"] = "1"
    res = run_bass_kernel_spmd(
        nc, in_maps, core_ids=list(range(N_CORES)),
        trace=want_trace,
    )
    last_results = res
    result = np.concatenate([r["result"] for r in res.results])
    polarity = np.ones(NG * (G - 1), dtype=np.float32)
    return result, polarity


# revision 19
# speedup vs baseline: 1.1183x; 1.1183x over previous
# Trainium2 Bass kernel for nn_DirectRanker (ragged_sequence).
#
# Math shortcut: result = tanh((sorted_enc[:,1:,:] - sorted_enc[:,:1,:]) @ W.T)
# commutes with the linear map, so per-row scores s = encodes @ W.T are
# computed FIRST (the memory-bound part: 512 MiB of bf16 streamed once), and
# the per-group sort/diff/tanh runs on the tiny [N] score vector:
#   result[g, k-1] = tanh(s_sorted[g, k] - s_sorted[g, 0]),  k = 1..63
#
# Sharding: groups split across 8 cores (2048 groups/core), no cross-core
# communication.
#
# v4: encodes is pre-arranged ON HOST to [2, 8, 128, 2, 8192] bf16
# (d-chunk, tile-pair, d, tile, col) so that every encode DMA is ONE dense
# 4 MiB linear HBM extent ([128, 16384] with partition stride 32 KiB) --
# linear extents measure 383 GB/s/core vs 308 for the strided layout.
# TensorE computes the matvec in its native orientation:
#   psum[1, 512] += W_chunk[128, 1].T @ ET_chunk[128, 512]
# (no on-chip transposes).  PSUM exits to SBUF fp16 split 3:1 between
# ScalarE and VectorE.  A single SBUF->SBUF DMA per 128-group tile
# relayouts the flat scores into [group(partition), elem(free)] for the
# sort.  Big streams ride the sync HWDGE ring; dependent small DMAs (score
# relayout, result store) ride the gpsimd SWDGE ring so they can't
# head-of-line block the encode stream.
#
# The sort chain for tile T is EMITTED one tile late (after tile T+1's
# matvec/exits).  Engine queues are strict FIFO: emitted in natural order,
# tile T's sort ops (which wait on the T score relayout) would sit in the
# Act/DVE/GpSimd queues AHEAD of tile T+1's psum exits and stall the whole
# psum -> matmul -> encode-DMA pipeline (measured: 8.5 us matmul stalls per
# pair, DMA duty 75%).  With the one-tile lag every emitted op's deps are
# already in flight, so the exits stream back-to-back.
#
# Exact stable argsort over y within each 64-row group: integer keys
#   key = (y * 2^23 + 2^23) * 64 | elem_index     (y is a multiple of 2^-23)
# are sorted through their f32 bitcast views (monotone for positive int32;
# keys lie in [2^29, 2^30) so the views are normal floats) with 8 rounds of
# DVE max8 + match_replace; perm = low 6 bits of the sorted keys. The score
# permutation runs on gpsimd local_scatter (fp16 as int16), ranks coming
# from scattering a descending iota by perm.
import os
from contextlib import ExitStack

import ml_dtypes
import numpy as np

import concourse.bacc as bacc
import concourse.mybir as mybir
import concourse.tile as tile
from concourse.bass_utils import run_bass_kernel_spmd

N_CORES = 8
N = 1048576
D = 256
G = 64
NG = N // G                # 16384 groups
ROWS = N // N_CORES        # 131072 rows per core
GPC = NG // N_CORES        # 2048 groups per core
T_TILES = GPC // 128       # 16 tiles of 128 groups (8192 rows) per core
PAIRS = T_TILES // 2       # 8 DMA pairs (2 tiles per 4 MiB extent)
RPT = 128 * G              # rows per tile = 8192
MM_N = 512                 # moving free size per matmul (1 psum bank)
EXIT_N = 2048              # scores per psum exit copy (4 psum banks)
F32 = mybir.dt.float32
F16 = mybir.dt.float16
BF16 = mybir.dt.bfloat16
I32 = mybir.dt.int32
I16 = mybir.dt.int16
Alu = mybir.AluOpType
Act = mybir.ActivationFunctionType

_built = {}


def _build_nc():
    nc = bacc.Bacc("TRN2", target_bir_lowering=False, debug=False,
                   num_devices=N_CORES)
    # host-prearranged encodes: [d-chunk, pair, d, tile-in-pair, col]
    et_in = nc.dram_tensor("et", [2, PAIRS, 128, 2, RPT], BF16,
                           kind="ExternalInput")
    y_in = nc.dram_tensor("y_coord", [ROWS], F32, kind="ExternalInput")
    w_in = nc.dram_tensor("w", [1, D], F32, kind="ExternalInput")
    out = nc.dram_tensor("result", [GPC * (G - 1)], F32, kind="ExternalOutput")

    out_r = out.ap().rearrange("(t p k) -> t p k", p=128, k=G - 1)

    with tile.TileContext(nc) as tc, ExitStack() as ctx:
        const_pool = ctx.enter_context(tc.tile_pool(name="const", bufs=1))
        epool = ctx.enter_context(tc.tile_pool(name="e", bufs=2))
        sfpool = ctx.enter_context(tc.tile_pool(name="sf", bufs=3))
        spool = ctx.enter_context(tc.tile_pool(name="s", bufs=6))
        scr_pool = ctx.enter_context(tc.tile_pool(name="scr", bufs=3))
        ps_pool = ctx.enter_context(
            tc.tile_pool(name="ps", bufs=2, space="PSUM"))

        # W FIRST on the (otherwise idle) scalar HWDGE ring so it lands in
        # ~2 us -- on the sync ring it would queue behind 8 MiB of encodes
        # and the first LDWEIGHTS would stall ~50 us.
        wsb = const_pool.tile([128, 2], F32)
        nc.scalar.dma_start(wsb[:],
                            w_in.ap()[0, :].rearrange("(c p) -> p c", p=128))
        wsb_h = const_pool.tile([128, 2], BF16)
        nc.vector.tensor_copy(wsb_h[:], wsb[:])

        etc_tiles = {}

        def load_pair(pair):
            etc0 = epool.tile([128, 2 * RPT], BF16, tag="etc0")
            nc.sync.dma_start(
                etc0[:], et_in.ap()[0, pair].rearrange("p t n -> p (t n)"))
            etc1 = epool.tile([128, 2 * RPT], BF16, tag="etc1")
            nc.sync.dma_start(
                etc1[:], et_in.ap()[1, pair].rearrange("p t n -> p (t n)"))
            etc_tiles[pair] = (etc0, etc1)

        load_pair(0)
        # all of y for this core, loaded up-front on the gpsimd SWDGE ring
        # (so its 2048 small descriptors can't head-of-line block the encode
        # stream on the sync ring): y_all[p, T, u] = y[(T*128+p)*64 + u]
        y_all = const_pool.tile([128, T_TILES, G], F32)
        nc.gpsimd.dma_start(
            y_all[:], y_in.ap().rearrange("(t p u) -> p t u", p=128, u=G))
        # free-dim iota (elem index within group) for the sort keys
        iota_i = const_pool.tile([128, G], I32)
        nc.gpsimd.iota(iota_i[:], pattern=[[1, G]], base=0, channel_multiplier=0)
        # descending iota (63..0) as int16: data for the rank-producing scatter
        iota_d16 = const_pool.tile([128, G], I16)
        nc.gpsimd.iota(iota_d16[:], pattern=[[-1, G]], base=G - 1,
                       channel_multiplier=0)

        sflat_tiles = {}
        s_tiles = {}
        rank_tiles = {}
        ss_tiles = {}
        th_tiles = {}

        def rank_chain(T):
            """Keys + sort + rank for tile T.  Depends ONLY on y_all (loaded
            at kernel start), so these DVE/gpsimd ops never wait and can be
            emitted arbitrarily early."""
            # keys on DVE: k64 = int(y*2^29 + 2^29) (exact in fp32: y is a
            # multiple of 2^-23, so y*2^29 = 64k and 2^29 + 64k have <= 24
            # significant bits over a 2^6 ulp); keys = k64 | elem_index
            k64 = spool.tile([128, G], I32, tag="k64")
            nc.vector.tensor_scalar(out=k64[:], in0=y_all[:, T, :],
                                    scalar1=float(1 << 29),
                                    scalar2=float(1 << 29),
                                    op0=Alu.mult, op1=Alu.add)
            keys = spool.tile([128, G], I32, tag="keys")
            nc.vector.tensor_tensor(out=keys[:], in0=k64[:], in1=iota_i[:],
                                    op=Alu.bitwise_or)

            # full descending sort of the int keys on DVE (f32 bitcast views)
            sorted_i = spool.tile([128, G], I32, tag="sorted")
            wka = scr_pool.tile([128, G], I32, tag="wka")
            wkb = scr_pool.tile([128, G], I32, tag="wkb")
            src = keys
            dst_t = wka
            for r in range(8):
                nc.vector.max(sorted_i[:, r * 8:(r + 1) * 8].bitcast(F32),
                              src[:].bitcast(F32))
                if r < 7:
                    nc.vector.match_replace(
                        dst_t[:].bitcast(F32),
                        sorted_i[:, r * 8:(r + 1) * 8].bitcast(F32),
                        src[:].bitcast(F32), 0.0)
                    src, dst_t = dst_t, (wkb if dst_t is wka else wka)

            # perm (descending argsort) = low 6 bits; int16 for the scatter
            perm32 = scr_pool.tile([128, G], I32, tag="perm32")
            nc.vector.tensor_scalar(out=perm32[:], in0=sorted_i[:],
                                    scalar1=63, scalar2=None,
                                    op0=Alu.bitwise_and)
            perm16 = spool.tile([128, G], I16, tag="perm16")
            nc.vector.tensor_copy(perm16[:], perm32[:])
            # rank_asc[i] = position of element i in ascending order
            rank16 = spool.tile([128, G], I16, tag="rank16")
            nc.gpsimd.local_scatter(rank16[:], iota_d16[:], perm16[:],
                                    channels=128, num_elems=G, num_idxs=G)
            rank_tiles[T] = rank16

        def score_scatter(T):
            """Permute tile T's fp16 scores by rank in one gpsimd scatter
            (rank16 and s_t both long since materialized)."""
            s_t = s_tiles.pop(T)
            rank16 = rank_tiles.pop(T)
            ssort = spool.tile([128, G], I16, tag="ssort")
            nc.gpsimd.local_scatter(ssort[:], s_t[:].bitcast(I16), rank16[:],
                                    channels=128, num_elems=G, num_idxs=G)
            ss_tiles[T] = ssort

        def tanh_store(T):
            """negs0 + tanh on Act (ssort long done -> no Act stall)."""
            ssort = ss_tiles.pop(T)
            ssf = ssort[:].bitcast(F16)
            negs0 = spool.tile([128, 1], F32, tag="negs0")
            nc.scalar.mul(negs0[:], ssf[:, 0:1], -1.0)
            th = spool.tile([128, G - 1], F32, tag="th")
            nc.scalar.activation(th[:], ssf[:, 1:G], Act.Tanh,
                                 bias=negs0[:], scale=1.0)
            th_tiles[T] = th

        def store_out(T):
            nc.gpsimd.dma_start(out_r[T], th_tiles.pop(T))

        # rank chains for the first two tiles run during the DMA head while
        # DVE/gpsimd are otherwise idle
        rank_chain(0)
        rank_chain(1)

        for pair in range(PAIRS):
            if pair > 0:
                load_pair(pair)
            etc0, etc1 = etc_tiles.pop(pair)

            for t in range(2):
                T = pair * 2 + t
                # ---- scores for tile T (native-orientation PE matvec) ----
                sflat = sfpool.tile([1, RPT], F16, tag="sflat")
                for j in range(RPT // EXIT_N):
                    ps = ps_pool.tile([1, EXIT_N], F32, tag="ps")
                    for q in range(EXIT_N // MM_N):
                        c0 = t * RPT + j * EXIT_N + q * MM_N
                        nc.tensor.matmul(ps[:, q * MM_N:(q + 1) * MM_N],
                                         wsb_h[:, 0:1], etc0[:, c0:c0 + MM_N],
                                         start=True, stop=False)
                        nc.tensor.matmul(ps[:, q * MM_N:(q + 1) * MM_N],
                                         wsb_h[:, 1:2], etc1[:, c0:c0 + MM_N],
                                         start=False, stop=True)
                    # exit psum -> sbuf fp16, 3:1 Act : DVE
                    dst = sflat[:, j * EXIT_N:(j + 1) * EXIT_N]
                    if j == 1:
                        nc.vector.tensor_copy(dst, ps[:])
                    else:
                        nc.scalar.copy(dst, ps[:])

                sflat_tiles[T] = sflat

                # lagged, decoupled tail stages: every op emitted below has
                # its deps already complete when its engine reaches it, so
                # no queue ever head-of-line blocks the psum exits.
                if T >= 1:
                    # relayout flat scores -> [group(partition), elem]; the
                    # gpsimd-ring issue is lagged one tile so it never waits
                    # on the exits
                    s_t = spool.tile([128, G], F16, tag="s")
                    nc.gpsimd.dma_start(s_t[:], sflat_tiles.pop(T - 1)[:])
                    s_tiles[T - 1] = s_t
                if T + 2 < T_TILES:
                    rank_chain(T + 2)
                if T >= 2:
                    score_scatter(T - 2)
                if T >= 3:
                    tanh_store(T - 3)
                if T >= 4:
                    store_out(T - 4)
        TL = T_TILES - 1
        s_t = spool.tile([128, G], F16, tag="s")
        nc.gpsimd.dma_start(s_t[:], sflat_tiles.pop(TL)[:])
        s_tiles[TL] = s_t
        score_scatter(TL - 1)
        score_scatter(TL)
        for T in (TL - 2, TL - 1, TL):
            tanh_store(T)
        for T in (TL - 3, TL - 2, TL - 1, TL):
            store_out(T)

    nc.compile()
    return nc


last_results = None


def kernel(encodes, y_coord, W, x_coord=None):
    global last_results
    if "nc" not in _built:
        _built["nc"] = _build_nc()
    nc = _built["nc"]

    e16 = np.asarray(encodes).astype(ml_dtypes.bfloat16)
    y_coord = np.ascontiguousarray(np.asarray(y_coord, dtype=np.float32))
    W = np.ascontiguousarray(np.asarray(W, dtype=np.float32))

    in_maps = []
    for c in range(N_CORES):
        # [ROWS, 256] -> [256, ROWS] -> [2, 128, 8, 2, 8192] (c d pair t n)
        # -> [2, 8, 128, 2, 8192] (c pair d t n), each [pair] slice one
        # dense 4 MiB extent per d-chunk
        et_c = np.ascontiguousarray(
            e16[c * ROWS:(c + 1) * ROWS].T
            .reshape(2, 128, PAIRS, 2, RPT)
            .transpose(0, 2, 1, 3, 4))
        in_maps.append({
            "et": et_c,
            "y_coord": y_coord[c * ROWS:(c + 1) * ROWS],
            "w": W,
        })
    # Only request tracing when the axon NTFF hook is importable; otherwise
    # force it off (bass_utils would crash importing antenv.axon_hooks if
    # BASS_TRACE leaked into the environment without the shim installed).
    want_trace = bool(os.environ.get("BASS_TRACE"))
    if want_trace:
        try:
            from antenv.axon_hooks import get_axon_ntff_profile_hook  # noqa: F401
        except ImportError:
            want_trace = False
            os.environ["BASS_NEVER_TRACE"] = "1"
    res = run_bass_kernel_spmd(
        nc, in_maps, core_ids=list(range(N_CORES)),
        trace=want_trace,
    )
    last_results = res
    result = np.concatenate([r["result"] for r in res.results])
    polarity = np.ones(NG * (G - 1), dtype=np.float32)
    return result, polarity


# revision 24
# speedup vs baseline: 1.2485x; 1.1164x over previous
# Trainium2 Bass kernel for nn_DirectRanker (ragged_sequence).
#
# Math shortcut: result = tanh((sorted_enc[:,1:,:] - sorted_enc[:,:1,:]) @ W.T)
# commutes with the linear map, so per-row scores s = encodes @ W.T are
# computed FIRST (the memory-bound part: 512 MiB of bf16 streamed once), and
# the per-group sort/diff/tanh runs on the tiny [N] score vector:
#   result[g, k-1] = tanh(s_sorted[g, k] - s_sorted[g, 0]),  k = 1..63
#
# Sharding: groups split across 8 cores (2048 groups/core), no cross-core
# communication.
#
# v4: encodes is pre-arranged ON HOST to [2, 8, 128, 2, 8192] bf16
# (d-chunk, tile-pair, d, tile, col) so that every encode DMA is ONE dense
# 4 MiB linear HBM extent ([128, 16384] with partition stride 32 KiB) --
# linear extents measure 383 GB/s/core vs 308 for the strided layout.
# TensorE computes the matvec in its native orientation:
#   psum[1, 512] += W_chunk[128, 1].T @ ET_chunk[128, 512]
# (no on-chip transposes).  PSUM exits to SBUF fp16 split 3:1 between
# ScalarE and VectorE.  A single SBUF->SBUF DMA per 128-group tile
# relayouts the flat scores into [group(partition), elem(free)] for the
# sort.  Big streams ride the sync HWDGE ring; dependent small DMAs (score
# relayout, result store) ride the gpsimd SWDGE ring so they can't
# head-of-line block the encode stream.
#
# The sort chain for tile T is EMITTED one tile late (after tile T+1's
# matvec/exits).  Engine queues are strict FIFO: emitted in natural order,
# tile T's sort ops (which wait on the T score relayout) would sit in the
# Act/DVE/GpSimd queues AHEAD of tile T+1's psum exits and stall the whole
# psum -> matmul -> encode-DMA pipeline (measured: 8.5 us matmul stalls per
# pair, DMA duty 75%).  With the one-tile lag every emitted op's deps are
# already in flight, so the exits stream back-to-back.
#
# Exact stable argsort over y within each 64-row group: integer keys
#   key = (y * 2^23 + 2^23) * 64 | elem_index     (y is a multiple of 2^-23)
# are sorted through their f32 bitcast views (monotone for positive int32;
# keys lie in [2^29, 2^30) so the views are normal floats) with 8 rounds of
# DVE max8 + match_replace; perm = low 6 bits of the sorted keys. The score
# permutation runs on gpsimd local_scatter (fp16 as int16), ranks coming
# from scattering a descending iota by perm.
import os
from contextlib import ExitStack

import ml_dtypes
import numpy as np

import concourse.bacc as bacc
import concourse.mybir as mybir
import concourse.tile as tile
from concourse.bass_utils import run_bass_kernel_spmd

N_CORES = 8
N = 1048576
D = 256
G = 64
NG = N // G                # 16384 groups
ROWS = N // N_CORES        # 131072 rows per core
GPC = NG // N_CORES        # 2048 groups per core
T_TILES = GPC // 128       # 16 tiles of 128 groups (8192 rows) per core
PAIRS = T_TILES // 2       # 8 DMA pairs (2 tiles per 4 MiB extent)
RPT = 128 * G              # rows per tile = 8192
MM_N = 512                 # moving free size per matmul (1 psum bank)
EXIT_N = 2048              # scores per psum exit copy (4 psum banks)
F32 = mybir.dt.float32
F16 = mybir.dt.float16
BF16 = mybir.dt.bfloat16
I32 = mybir.dt.int32
I16 = mybir.dt.int16
Alu = mybir.AluOpType
Act = mybir.ActivationFunctionType

_built = {}


def _build_nc():
    nc = bacc.Bacc("TRN2", target_bir_lowering=False, debug=False,
                   num_devices=N_CORES)
    # host-prearranged encodes: [d-chunk, pair, d, tile-in-pair, col]
    et_in = nc.dram_tensor("et", [2, PAIRS, 128, 2, RPT], BF16,
                           kind="ExternalInput")
    y_in = nc.dram_tensor("y_coord", [ROWS], F32, kind="ExternalInput")
    w_in = nc.dram_tensor("w", [1, D], F32, kind="ExternalInput")
    out = nc.dram_tensor("result", [GPC * (G - 1)], F32, kind="ExternalOutput")

    out_r = out.ap().rearrange("(t p k) -> t p k", p=128, k=G - 1)

    with tile.TileContext(nc) as tc, ExitStack() as ctx:
        const_pool = ctx.enter_context(tc.tile_pool(name="const", bufs=1))
        epool = ctx.enter_context(tc.tile_pool(name="e", bufs=2))
        sfpool = ctx.enter_context(tc.tile_pool(name="sf", bufs=3))
        spool = ctx.enter_context(tc.tile_pool(name="s", bufs=6))
        scr_pool = ctx.enter_context(tc.tile_pool(name="scr", bufs=3))
        ps_pool = ctx.enter_context(
            tc.tile_pool(name="ps", bufs=2, space="PSUM"))

        # W FIRST on the (otherwise idle) scalar HWDGE ring so it lands in
        # ~2 us -- on the sync ring it would queue behind 8 MiB of encodes
        # and the first LDWEIGHTS would stall ~50 us.
        wsb = const_pool.tile([128, 2], F32)
        nc.scalar.dma_start(wsb[:],
                            w_in.ap()[0, :].rearrange("(c p) -> p c", p=128))
        wsb_h = const_pool.tile([128, 2], BF16)
        nc.vector.tensor_copy(wsb_h[:], wsb[:])

        etc_tiles = {}

        def load_pair(pair):
            etc0 = epool.tile([128, 2 * RPT], BF16, tag="etc0")
            nc.sync.dma_start(
                etc0[:], et_in.ap()[0, pair].rearrange("p t n -> p (t n)"))
            etc1 = epool.tile([128, 2 * RPT], BF16, tag="etc1")
            nc.sync.dma_start(
                etc1[:], et_in.ap()[1, pair].rearrange("p t n -> p (t n)"))
            etc_tiles[pair] = (etc0, etc1)

        load_pair(0)
        # all of y for this core, loaded up-front on the scalar ring (Act is
        # idle at start; the gpsimd ring would force a SWDGE<->scatter lib
        # reload on GpSimd, the sync ring would delay the encode stream):
        # y_all[p, T, u] = y[(T*128+p)*64 + u]
        y_all = const_pool.tile([128, T_TILES, G], F32)
        nc.scalar.dma_start(
            y_all[:], y_in.ap().rearrange("(t p u) -> p t u", p=128, u=G))
        # free-dim iota (elem index within group) for the sort keys
        iota_i = const_pool.tile([128, G], I32)
        nc.gpsimd.iota(iota_i[:], pattern=[[1, G]], base=0, channel_multiplier=0)
        # descending iota (63..0) as int16: data for the rank-producing scatter
        iota_d16 = const_pool.tile([128, G], I16)
        nc.gpsimd.iota(iota_d16[:], pattern=[[-1, G]], base=G - 1,
                       channel_multiplier=0)

        sflat_tiles = {}
        s_tiles = {}
        rank_tiles = {}
        ss_tiles = {}
        th_tiles = {}

        def rank_chain(T):
            """Keys + sort + rank for tile T.  Depends ONLY on y_all (loaded
            at kernel start), so these DVE/gpsimd ops never wait and can be
            emitted arbitrarily early."""
            # keys on DVE: k64 = int(y*2^29 + 2^29) (exact in fp32: y is a
            # multiple of 2^-23, so y*2^29 = 64k and 2^29 + 64k have <= 24
            # significant bits over a 2^6 ulp); keys = k64 | elem_index
            k64 = spool.tile([128, G], I32, tag="k64")
            nc.vector.tensor_scalar(out=k64[:], in0=y_all[:, T, :],
                                    scalar1=float(1 << 29),
                                    scalar2=float(1 << 29),
                                    op0=Alu.mult, op1=Alu.add)
            keys = spool.tile([128, G], I32, tag="keys")
            nc.vector.tensor_tensor(out=keys[:], in0=k64[:], in1=iota_i[:],
                                    op=Alu.bitwise_or)

            # full descending sort of the int keys on DVE (f32 bitcast views)
            sorted_i = spool.tile([128, G], I32, tag="sorted")
            wka = scr_pool.tile([128, G], I32, tag="wka")
            wkb = scr_pool.tile([128, G], I32, tag="wkb")
            src = keys
            dst_t = wka
            for r in range(8):
                nc.vector.max(sorted_i[:, r * 8:(r + 1) * 8].bitcast(F32),
                              src[:].bitcast(F32))
                if r < 7:
                    nc.vector.match_replace(
                        dst_t[:].bitcast(F32),
                        sorted_i[:, r * 8:(r + 1) * 8].bitcast(F32),
                        src[:].bitcast(F32), 0.0)
                    src, dst_t = dst_t, (wkb if dst_t is wka else wka)

            # perm (descending argsort) = low 6 bits; int16 for the scatter
            perm32 = scr_pool.tile([128, G], I32, tag="perm32")
            nc.vector.tensor_scalar(out=perm32[:], in0=sorted_i[:],
                                    scalar1=63, scalar2=None,
                                    op0=Alu.bitwise_and)
            perm16 = spool.tile([128, G], I16, tag="perm16")
            nc.vector.tensor_copy(perm16[:], perm32[:])
            # rank_asc[i] = position of element i in ascending order
            rank16 = spool.tile([128, G], I16, tag="rank16")
            nc.gpsimd.local_scatter(rank16[:], iota_d16[:], perm16[:],
                                    channels=128, num_elems=G, num_idxs=G)
            rank_tiles[T] = rank16

        def score_scatter(T):
            """Permute tile T's fp16 scores by rank in one gpsimd scatter
            (rank16 and s_t both long since materialized)."""
            s_t = s_tiles.pop(T)
            rank16 = rank_tiles.pop(T)
            ssort = spool.tile([128, G], I16, tag="ssort")
            nc.gpsimd.local_scatter(ssort[:], s_t[:].bitcast(I16), rank16[:],
                                    channels=128, num_elems=G, num_idxs=G)
            ss_tiles[T] = ssort

        def tanh_store(T):
            """negs0 + tanh on Act (ssort long done -> no Act stall)."""
            ssort = ss_tiles.pop(T)
            ssf = ssort[:].bitcast(F16)
            negs0 = spool.tile([128, 1], F32, tag="negs0")
            nc.scalar.mul(negs0[:], ssf[:, 0:1], -1.0)
            th = spool.tile([128, G - 1], F32, tag="th")
            nc.scalar.activation(th[:], ssf[:, 1:G], Act.Tanh,
                                 bias=negs0[:], scale=1.0)
            th_tiles[T] = th

        def store_out(T):
            # lagged far enough that tanh(T) long completed: the sync-ring
            # issue never waits, so it can't block the encode stream
            nc.sync.dma_start(out_r[T], th_tiles.pop(T))

        # rank chains for the first two tiles run during the DMA head while
        # DVE/gpsimd are otherwise idle
        rank_chain(0)
        rank_chain(1)

        for pair in range(PAIRS):
            if pair > 0:
                load_pair(pair)
            etc0, etc1 = etc_tiles.pop(pair)

            for t in range(2):
                T = pair * 2 + t
                # ---- scores for tile T (native-orientation PE matvec) ----
                sflat = sfpool.tile([1, RPT], F16, tag="sflat")
                for j in range(RPT // EXIT_N):
                    ps = ps_pool.tile([1, EXIT_N], F32, tag="ps")
                    for q in range(EXIT_N // MM_N):
                        c0 = t * RPT + j * EXIT_N + q * MM_N
                        nc.tensor.matmul(ps[:, q * MM_N:(q + 1) * MM_N],
                                         wsb_h[:, 0:1], etc0[:, c0:c0 + MM_N],
                                         start=True, stop=False)
                        nc.tensor.matmul(ps[:, q * MM_N:(q + 1) * MM_N],
                                         wsb_h[:, 1:2], etc1[:, c0:c0 + MM_N],
                                         start=False, stop=True)
                    # exit psum -> sbuf fp16.  ALL exits on Act: the psum
                    # slots are then freed by one strictly-FIFO engine whose
                    # queue holds nothing with unresolved deps -- DVE would
                    # add rank-chain jitter (4.6 us) to the slot path.
                    nc.scalar.copy(sflat[:, j * EXIT_N:(j + 1) * EXIT_N],
                                   ps[:])

                sflat_tiles[T] = sflat

                # lagged, decoupled tail stages: every op emitted below has
                # its deps already complete when its engine reaches it, so
                # no queue ever head-of-line blocks the psum exits.
                if T >= 1:
                    # relayout flat scores -> [group(partition), elem]; the
                    # sync-ring issue is lagged one tile so it never waits
                    # on the exits (and gpsimd stays scatter-only: mixing
                    # SWDGE DMAs with local_scatter forces a ~2.2 us gpsimd
                    # lib reload per switch)
                    s_t = spool.tile([128, G], F16, tag="s")
                    nc.sync.dma_start(s_t[:], sflat_tiles.pop(T - 1)[:])
                    s_tiles[T - 1] = s_t
                if T + 2 < T_TILES:
                    rank_chain(T + 2)
                if T >= 2:
                    score_scatter(T - 2)
                if T >= 3:
                    tanh_store(T - 3)
                if T >= 4:
                    store_out(T - 4)
        TL = T_TILES - 1
        s_t = spool.tile([128, G], F16, tag="s")
        nc.sync.dma_start(s_t[:], sflat_tiles.pop(TL)[:])
        s_tiles[TL] = s_t
        score_scatter(TL - 1)
        score_scatter(TL)
        for T in (TL - 2, TL - 1, TL):
            tanh_store(T)
        for T in (TL - 3, TL - 2, TL - 1, TL):
            store_out(T)

    nc.compile()
    return nc


last_results = None


def kernel(encodes, y_coord, W, x_coord=None):
    global last_results
    if "nc" not in _built:
        _built["nc"] = _build_nc()
    nc = _built["nc"]

    e16 = np.asarray(encodes).astype(ml_dtypes.bfloat16)
    y_coord = np.ascontiguousarray(np.asarray(y_coord, dtype=np.float32))
    W = np.ascontiguousarray(np.asarray(W, dtype=np.float32))

    in_maps = []
    for c in range(N_CORES):
        # [ROWS, 256] -> [256, ROWS] -> [2, 128, 8, 2, 8192] (c d pair t n)
        # -> [2, 8, 128, 2, 8192] (c pair d t n), each [pair] slice one
        # dense 4 MiB extent per d-chunk
        et_c = np.ascontiguousarray(
            e16[c * ROWS:(c + 1) * ROWS].T
            .reshape(2, 128, PAIRS, 2, RPT)
            .transpose(0, 2, 1, 3, 4))
        in_maps.append({
            "et": et_c,
            "y_coord": y_coord[c * ROWS:(c + 1) * ROWS],
            "w": W,
        })
    # Only request tracing when the axon NTFF hook is importable; otherwise
    # force it off (bass_utils would crash importing antenv.axon_hooks if
    # BASS_TRACE leaked into the environment without the shim installed).
    want_trace = bool(os.environ.get("BASS_TRACE"))
    if want_trace:
        try:
            from antenv.axon_hooks import get_axon_ntff_profile_hook  # noqa: F401
        except ImportError:
            want_trace = False
            os.environ["BASS_NEVER_TRACE"] = "1"
    res = run_bass_kernel_spmd(
        nc, in_maps, core_ids=list(range(N_CORES)),
        trace=want_trace,
    )
    last_results = res
    result = np.concatenate([r["result"] for r in res.results])
    polarity = np.ones(NG * (G - 1), dtype=np.float32)
    return result, polarity


# revision 33
# speedup vs baseline: 1.3704x; 1.0977x over previous
# Trainium2 Bass kernel for nn_DirectRanker (ragged_sequence).
#
# Math shortcut: result = tanh((sorted_enc[:,1:,:] - sorted_enc[:,:1,:]) @ W.T)
# commutes with the linear map, so per-row scores s = encodes @ W.T are
# computed FIRST (the memory-bound part: 512 MiB of bf16 streamed once), and
# the per-group sort/diff/tanh runs on the tiny [N] score vector:
#   result[g, k-1] = tanh(s_sorted[g, k] - s_sorted[g, 0]),  k = 1..63
#
# Sharding: groups split across 8 cores (2048 groups/core), no cross-core
# communication.
#
# v4: encodes is pre-arranged ON HOST to [2, 8, 128, 2, 8192] bf16
# (d-chunk, tile-pair, d, tile, col) so that every encode DMA is ONE dense
# 4 MiB linear HBM extent ([128, 16384] with partition stride 32 KiB) --
# linear extents measure 383 GB/s/core vs 308 for the strided layout.
# TensorE computes the matvec in its native orientation:
#   psum[1, 512] += W_chunk[128, 1].T @ ET_chunk[128, 512]
# (no on-chip transposes).  PSUM exits to SBUF fp16 split 3:1 between
# ScalarE and VectorE.  A single SBUF->SBUF DMA per 128-group tile
# relayouts the flat scores into [group(partition), elem(free)] for the
# sort.  Big streams ride the sync HWDGE ring; dependent small DMAs (score
# relayout, result store) ride the gpsimd SWDGE ring so they can't
# head-of-line block the encode stream.
#
# The sort chain for tile T is EMITTED one tile late (after tile T+1's
# matvec/exits).  Engine queues are strict FIFO: emitted in natural order,
# tile T's sort ops (which wait on the T score relayout) would sit in the
# Act/DVE/GpSimd queues AHEAD of tile T+1's psum exits and stall the whole
# psum -> matmul -> encode-DMA pipeline (measured: 8.5 us matmul stalls per
# pair, DMA duty 75%).  With the one-tile lag every emitted op's deps are
# already in flight, so the exits stream back-to-back.
#
# Exact stable argsort over y within each 64-row group: integer keys
#   key = (y * 2^23 + 2^23) * 64 | elem_index     (y is a multiple of 2^-23)
# are sorted through their f32 bitcast views (monotone for positive int32;
# keys lie in [2^29, 2^30) so the views are normal floats) with 8 rounds of
# DVE max8 + match_replace; perm = low 6 bits of the sorted keys. The score
# permutation runs on gpsimd local_scatter (fp16 as int16), ranks coming
# from scattering a descending iota by perm.
import os
from contextlib import ExitStack

import ml_dtypes
import numpy as np

import concourse.bacc as bacc
import concourse.mybir as mybir
import concourse.tile as tile
from concourse.bass_utils import run_bass_kernel_spmd

N_CORES = 8
N = 1048576
D = 256
G = 64
NG = N // G                # 16384 groups
ROWS = N // N_CORES        # 131072 rows per core
GPC = NG // N_CORES        # 2048 groups per core
T_TILES = GPC // 128       # 16 tiles of 128 groups (8192 rows) per core
PAIRS = T_TILES // 2       # 8 DMA pairs (2 tiles per 4 MiB extent)
RPT = 128 * G              # rows per tile = 8192
MM_N = 512                 # moving free size per matmul (1 psum bank)
EXIT_N = 1024              # scores per psum exit copy (2 psum banks)
F32 = mybir.dt.float32
F16 = mybir.dt.float16
BF16 = mybir.dt.bfloat16
I32 = mybir.dt.int32
I16 = mybir.dt.int16
Alu = mybir.AluOpType
Act = mybir.ActivationFunctionType

_built = {}


def _build_nc():
    nc = bacc.Bacc("TRN2", target_bir_lowering=False, debug=False,
                   num_devices=N_CORES)
    # host-prearranged encodes: [d-chunk, pair, d, tile-in-pair, col]
    et_in = nc.dram_tensor("et", [2, PAIRS, 128, 2, RPT], BF16,
                           kind="ExternalInput")
    y_in = nc.dram_tensor("y_coord", [ROWS], F32, kind="ExternalInput")
    w_in = nc.dram_tensor("w", [1, D], F32, kind="ExternalInput")
    out = nc.dram_tensor("result", [GPC * (G - 1)], F32, kind="ExternalOutput")

    out_r = out.ap().rearrange("(t p k) -> t p k", p=128, k=G - 1)

    with tile.TileContext(nc) as tc, ExitStack() as ctx:
        const_pool = ctx.enter_context(tc.tile_pool(name="const", bufs=1))
        epool = ctx.enter_context(tc.tile_pool(name="e", bufs=2))
        sfpool = ctx.enter_context(tc.tile_pool(name="sf", bufs=3))
        spool = ctx.enter_context(tc.tile_pool(name="s", bufs=6))
        scr_pool = ctx.enter_context(tc.tile_pool(name="scr", bufs=3))
        ps_pool = ctx.enter_context(
            tc.tile_pool(name="ps", bufs=4, space="PSUM"))

        # W FIRST on the (otherwise idle) scalar HWDGE ring so it lands in
        # ~2 us -- on the sync ring it would queue behind 8 MiB of encodes
        # and the first LDWEIGHTS would stall ~50 us.
        wsb = const_pool.tile([128, 2], F32)
        nc.scalar.dma_start(wsb[:],
                            w_in.ap()[0, :].rearrange("(c p) -> p c", p=128))
        wsb_h = const_pool.tile([128, 2], BF16)
        nc.vector.tensor_copy(wsb_h[:], wsb[:])

        etc_tiles = {}

        def load_pair(pair):
            etc0 = epool.tile([128, 2 * RPT], BF16, tag="etc0")
            nc.sync.dma_start(
                etc0[:], et_in.ap()[0, pair].rearrange("p t n -> p (t n)"))
            etc1 = epool.tile([128, 2 * RPT], BF16, tag="etc1")
            nc.sync.dma_start(
                etc1[:], et_in.ap()[1, pair].rearrange("p t n -> p (t n)"))
            etc_tiles[pair] = (etc0, etc1)

        # y is loaded PER TILE (lagged several tiles ahead on the sync
        # ring): one big upfront y DMA has 2048 tiny 256 B descriptors that
        # steal ~20 us of SDMA bandwidth from the encode stream at the start.
        y_r = y_in.ap().rearrange("(t p u) -> t p u", p=128, u=G)
        y_tiles = {}

        def load_y(T):
            y_t = spool.tile([128, G], F32, tag="y")
            nc.sync.dma_start(y_t[:], y_r[T])
            y_tiles[T] = y_t

        # first y tiles ahead of the encode stream (128 KiB, ~0.4 us) so the
        # head-time rank chains have their input immediately
        for T in range(4):
            load_y(T)
        load_pair(0)
        # free-dim iota (elem index within group) for the sort keys
        iota_i = const_pool.tile([128, G], I32)
        nc.gpsimd.iota(iota_i[:], pattern=[[1, G]], base=0, channel_multiplier=0)
        # descending iota (63..0) as int16: data for the rank-producing scatter
        iota_d16 = const_pool.tile([128, G], I16)
        nc.gpsimd.iota(iota_d16[:], pattern=[[-1, G]], base=G - 1,
                       channel_multiplier=0)

        sflat_tiles = {}
        s_tiles = {}
        rank_tiles = {}
        ss_tiles = {}
        th_tiles = {}

        def rank_chain(T):
            """Keys + sort + rank for tile T.  Depends ONLY on y_all (loaded
            at kernel start), so these DVE/gpsimd ops never wait and can be
            emitted arbitrarily early."""
            # keys on DVE: k64 = int(y*2^29 + 2^29) (exact in fp32: y is a
            # multiple of 2^-23, so y*2^29 = 64k and 2^29 + 64k have <= 24
            # significant bits over a 2^6 ulp); keys = k64 | elem_index
            y_t = y_tiles.pop(T)
            k64 = spool.tile([128, G], I32, tag="k64")
            nc.vector.tensor_scalar(out=k64[:], in0=y_t[:],
                                    scalar1=float(1 << 29),
                                    scalar2=float(1 << 29),
                                    op0=Alu.mult, op1=Alu.add)
            keys = spool.tile([128, G], I32, tag="keys")
            nc.vector.tensor_tensor(out=keys[:], in0=k64[:], in1=iota_i[:],
                                    op=Alu.bitwise_or)

            # full descending sort of the int keys on DVE (f32 bitcast views)
            sorted_i = spool.tile([128, G], I32, tag="sorted")
            wka = scr_pool.tile([128, G], I32, tag="wka")
            wkb = scr_pool.tile([128, G], I32, tag="wkb")
            src = keys
            dst_t = wka
            for r in range(8):
                nc.vector.max(sorted_i[:, r * 8:(r + 1) * 8].bitcast(F32),
                              src[:].bitcast(F32))
                if r < 7:
                    nc.vector.match_replace(
                        dst_t[:].bitcast(F32),
                        sorted_i[:, r * 8:(r + 1) * 8].bitcast(F32),
                        src[:].bitcast(F32), 0.0)
                    src, dst_t = dst_t, (wkb if dst_t is wka else wka)

            # perm (descending argsort) = low 6 bits; int16 for the scatter
            perm32 = scr_pool.tile([128, G], I32, tag="perm32")
            nc.vector.tensor_scalar(out=perm32[:], in0=sorted_i[:],
                                    scalar1=63, scalar2=None,
                                    op0=Alu.bitwise_and)
            perm16 = spool.tile([128, G], I16, tag="perm16")
            nc.vector.tensor_copy(perm16[:], perm32[:])
            # rank_asc[i] = position of element i in ascending order
            rank16 = spool.tile([128, G], I16, tag="rank16")
            nc.gpsimd.local_scatter(rank16[:], iota_d16[:], perm16[:],
                                    channels=128, num_elems=G, num_idxs=G)
            rank_tiles[T] = rank16

        def score_scatter(T):
            """Permute tile T's fp16 scores by rank in one gpsimd scatter
            (rank16 and s_t both long since materialized)."""
            s_t = s_tiles.pop(T)
            rank16 = rank_tiles.pop(T)
            ssort = spool.tile([128, G], I16, tag="ssort")
            nc.gpsimd.local_scatter(ssort[:], s_t[:].bitcast(I16), rank16[:],
                                    channels=128, num_elems=G, num_idxs=G)
            ss_tiles[T] = ssort

        def tanh_store(T):
            """negs0 + tanh on Act (ssort long done -> no Act stall)."""
            ssort = ss_tiles.pop(T)
            ssf = ssort[:].bitcast(F16)
            negs0 = spool.tile([128, 1], F32, tag="negs0")
            nc.scalar.mul(negs0[:], ssf[:, 0:1], -1.0)
            th = spool.tile([128, G - 1], F32, tag="th")
            nc.scalar.activation(th[:], ssf[:, 1:G], Act.Tanh,
                                 bias=negs0[:], scale=1.0)
            th_tiles[T] = th

        def store_out(T):
            # lagged far enough that tanh(T) long completed: the sync-ring
            # issue never waits, so it can't block the encode stream
            nc.sync.dma_start(out_r[T], th_tiles.pop(T))

        # rank chains for the first two tiles run during the DMA head while
        # DVE/gpsimd are otherwise idle
        rank_chain(0)
        rank_chain(1)

        for pair in range(PAIRS):
            if pair > 0:
                load_pair(pair)
            etc0, etc1 = etc_tiles.pop(pair)

            for t in range(2):
                T = pair * 2 + t
                # ---- scores for tile T (native-orientation PE matvec) ----
                sflat = sfpool.tile([1, RPT], F16, tag="sflat")
                for j in range(RPT // EXIT_N):
                    ps = ps_pool.tile([1, EXIT_N], F32, tag="ps")
                    for q in range(EXIT_N // MM_N):
                        c0 = t * RPT + j * EXIT_N + q * MM_N
                        nc.tensor.matmul(ps[:, q * MM_N:(q + 1) * MM_N],
                                         wsb_h[:, 0:1], etc0[:, c0:c0 + MM_N],
                                         start=True, stop=False)
                        nc.tensor.matmul(ps[:, q * MM_N:(q + 1) * MM_N],
                                         wsb_h[:, 1:2], etc1[:, c0:c0 + MM_N],
                                         start=False, stop=True)
                    # exit psum -> sbuf fp16.  ALL exits on Act: the psum
                    # slots are then freed by one strictly-FIFO engine whose
                    # queue holds nothing with unresolved deps -- DVE would
                    # add rank-chain jitter (4.6 us) to the slot path.
                    nc.scalar.copy(sflat[:, j * EXIT_N:(j + 1) * EXIT_N],
                                   ps[:])

                sflat_tiles[T] = sflat

                # lagged, decoupled tail stages: every op emitted below has
                # its deps already complete when its engine reaches it, so
                # no queue ever head-of-line blocks the psum exits.
                if T >= 1:
                    # relayout flat scores -> [group(partition), elem]; the
                    # sync-ring issue is lagged one tile so it never waits
                    # on the exits (and gpsimd stays scatter-only: mixing
                    # SWDGE DMAs with local_scatter forces a ~2.2 us gpsimd
                    # lib reload per switch)
                    s_t = spool.tile([128, G], F16, tag="s")
                    nc.sync.dma_start(s_t[:], sflat_tiles.pop(T - 1)[:])
                    s_tiles[T - 1] = s_t
                if T + 4 < T_TILES:
                    load_y(T + 4)
                if T + 2 < T_TILES:
                    rank_chain(T + 2)
                if T >= 2:
                    score_scatter(T - 2)
                if T >= 3:
                    tanh_store(T - 3)
                if T >= 4:
                    store_out(T - 4)
        TL = T_TILES - 1
        s_t = spool.tile([128, G], F16, tag="s")
        nc.sync.dma_start(s_t[:], sflat_tiles.pop(TL)[:])
        s_tiles[TL] = s_t
        score_scatter(TL - 1)
        score_scatter(TL)
        for T in (TL - 2, TL - 1, TL):
            tanh_store(T)
        for T in (TL - 3, TL - 2, TL - 1, TL):
            store_out(T)

    nc.compile()
    return nc


last_results = None


def kernel(encodes, y_coord, W, x_coord=None):
    global last_results
    if "nc" not in _built:
        _built["nc"] = _build_nc()
    nc = _built["nc"]

    e16 = np.asarray(encodes).astype(ml_dtypes.bfloat16)
    y_coord = np.ascontiguousarray(np.asarray(y_coord, dtype=np.float32))
    W = np.ascontiguousarray(np.asarray(W, dtype=np.float32))

    in_maps = []
    for c in range(N_CORES):
        # [ROWS, 256] -> [256, ROWS] -> [2, 128, 8, 2, 8192] (c d pair t n)
        # -> [2, 8, 128, 2, 8192] (c pair d t n), each [pair] slice one
        # dense 4 MiB extent per d-chunk
        et_c = np.ascontiguousarray(
            e16[c * ROWS:(c + 1) * ROWS].T
            .reshape(2, 128, PAIRS, 2, RPT)
            .transpose(0, 2, 1, 3, 4))
        in_maps.append({
            "et": et_c,
            "y_coord": y_coord[c * ROWS:(c + 1) * ROWS],
            "w": W,
        })
    # Only request tracing when the axon NTFF hook is importable; otherwise
    # force it off (bass_utils would crash importing antenv.axon_hooks if
    # BASS_TRACE leaked into the environment without the shim installed).
    want_trace = bool(os.environ.get("BASS_TRACE"))
    if want_trace:
        try:
            from antenv.axon_hooks import get_axon_ntff_profile_hook  # noqa: F401
        except ImportError:
            want_trace = False
            os.environ["BASS_NEVER_TRACE"] = "1"
    res = run_bass_kernel_spmd(
        nc, in_maps, core_ids=list(range(N_CORES)),
        trace=want_trace,
    )
    last_results = res
    result = np.concatenate([r["result"] for r in res.results])
    polarity = np.ones(NG * (G - 1), dtype=np.float32)
    return result, polarity


# revision 38
# speedup vs baseline: 1.4267x; 1.0411x over previous
# Trainium2 Bass kernel for nn_DirectRanker (ragged_sequence).
#
# Math shortcut: result = tanh((sorted_enc[:,1:,:] - sorted_enc[:,:1,:]) @ W.T)
# commutes with the linear map, so per-row scores s = encodes @ W.T are
# computed FIRST (the memory-bound part: 512 MiB of bf16 streamed once), and
# the per-group sort/diff/tanh runs on the tiny [N] score vector:
#   result[g, k-1] = tanh(s_sorted[g, k] - s_sorted[g, 0]),  k = 1..63
#
# Sharding: groups split across 8 cores (2048 groups/core), no cross-core
# communication.
#
# v4: encodes is pre-arranged ON HOST to [2, 8, 128, 2, 8192] bf16
# (d-chunk, tile-pair, d, tile, col) so that every encode DMA is ONE dense
# 4 MiB linear HBM extent ([128, 16384] with partition stride 32 KiB) --
# linear extents measure 383 GB/s/core vs 308 for the strided layout.
# TensorE computes the matvec in its native orientation:
#   psum[1, 512] += W_chunk[128, 1].T @ ET_chunk[128, 512]
# (no on-chip transposes).  PSUM exits to SBUF fp16 split 3:1 between
# ScalarE and VectorE.  A single SBUF->SBUF DMA per 128-group tile
# relayouts the flat scores into [group(partition), elem(free)] for the
# sort.  Big streams ride the sync HWDGE ring; dependent small DMAs (score
# relayout, result store) ride the gpsimd SWDGE ring so they can't
# head-of-line block the encode stream.
#
# The sort chain for tile T is EMITTED one tile late (after tile T+1's
# matvec/exits).  Engine queues are strict FIFO: emitted in natural order,
# tile T's sort ops (which wait on the T score relayout) would sit in the
# Act/DVE/GpSimd queues AHEAD of tile T+1's psum exits and stall the whole
# psum -> matmul -> encode-DMA pipeline (measured: 8.5 us matmul stalls per
# pair, DMA duty 75%).  With the one-tile lag every emitted op's deps are
# already in flight, so the exits stream back-to-back.
#
# Exact stable argsort over y within each 64-row group: integer keys
#   key = (y * 2^23 + 2^23) * 64 | elem_index     (y is a multiple of 2^-23)
# are sorted through their f32 bitcast views (monotone for positive int32;
# keys lie in [2^29, 2^30) so the views are normal floats) with 8 rounds of
# DVE max8 + match_replace; perm = low 6 bits of the sorted keys. The score
# permutation runs on gpsimd local_scatter (fp16 as int16), ranks coming
# from scattering a descending iota by perm.
import os
from contextlib import ExitStack

import ml_dtypes
import numpy as np

import concourse.bacc as bacc
import concourse.mybir as mybir
import concourse.tile as tile
from concourse.bass_utils import run_bass_kernel_spmd

N_CORES = 8
N = 1048576
D = 256
G = 64
NG = N // G                # 16384 groups
ROWS = N // N_CORES        # 131072 rows per core
GPC = NG // N_CORES        # 2048 groups per core
T_TILES = GPC // 128       # 16 tiles of 128 groups (8192 rows) per core
PAIRS = T_TILES // 2       # 8 DMA pairs (2 tiles per 4 MiB extent)
RPT = 128 * G              # rows per tile = 8192
MM_N = 512                 # moving free size per matmul (1 psum bank)
EXIT_N = 2048              # scores per psum exit copy (4 psum banks)
F32 = mybir.dt.float32
F16 = mybir.dt.float16
BF16 = mybir.dt.bfloat16
I32 = mybir.dt.int32
I16 = mybir.dt.int16
Alu = mybir.AluOpType
Act = mybir.ActivationFunctionType

_built = {}


def _build_nc():
    nc = bacc.Bacc("TRN2", target_bir_lowering=False, debug=False,
                   num_devices=N_CORES)
    # host-prearranged encodes: [d-chunk, pair, d, tile-in-pair, col]
    et_in = nc.dram_tensor("et", [2, PAIRS, 128, 2, RPT], BF16,
                           kind="ExternalInput")
    y_in = nc.dram_tensor("y_coord", [ROWS], F32, kind="ExternalInput")
    w_in = nc.dram_tensor("w", [1, D], F32, kind="ExternalInput")
    out = nc.dram_tensor("result", [GPC * (G - 1)], F32, kind="ExternalOutput")

    out_r = out.ap().rearrange("(t p k) -> t p k", p=128, k=G - 1)

    with tile.TileContext(nc) as tc, ExitStack() as ctx:
        const_pool = ctx.enter_context(tc.tile_pool(name="const", bufs=1))
        epool = ctx.enter_context(tc.tile_pool(name="e", bufs=2))
        sfpool = ctx.enter_context(tc.tile_pool(name="sf", bufs=3))
        spool = ctx.enter_context(tc.tile_pool(name="s", bufs=6))
        scr_pool = ctx.enter_context(tc.tile_pool(name="scr", bufs=3))
        ps_pool = ctx.enter_context(
            tc.tile_pool(name="ps", bufs=2, space="PSUM"))

        # W FIRST on the (otherwise idle) scalar HWDGE ring so it lands in
        # ~2 us -- on the sync ring it would queue behind 8 MiB of encodes
        # and the first LDWEIGHTS would stall ~50 us.
        wsb = const_pool.tile([128, 2], F32)
        nc.scalar.dma_start(wsb[:],
                            w_in.ap()[0, :].rearrange("(c p) -> p c", p=128))
        wsb_h = const_pool.tile([128, 2], BF16)
        nc.vector.tensor_copy(wsb_h[:], wsb[:])

        etc_tiles = {}

        def load_pair(pair, split=False):
            etc0 = epool.tile([128, 2 * RPT], BF16, tag="etc0")
            etc1 = epool.tile([128, 2 * RPT], BF16, tag="etc1")
            if split:
                # per-tile halves (strided extents, slightly slower DMA) so
                # the first tile's matmuls can start ~10 us earlier -- used
                # only for pair 0 at the pipeline head
                for t in range(2):
                    nc.sync.dma_start(etc0[:, t * RPT:(t + 1) * RPT],
                                      et_in.ap()[0, pair, :, t, :])
                    nc.sync.dma_start(etc1[:, t * RPT:(t + 1) * RPT],
                                      et_in.ap()[1, pair, :, t, :])
            else:
                nc.sync.dma_start(
                    etc0[:], et_in.ap()[0, pair].rearrange("p t n -> p (t n)"))
                nc.sync.dma_start(
                    etc1[:], et_in.ap()[1, pair].rearrange("p t n -> p (t n)"))
            etc_tiles[pair] = (etc0, etc1)

        # y is loaded PER TILE (lagged several tiles ahead on the sync
        # ring): one big upfront y DMA has 2048 tiny 256 B descriptors that
        # steal ~20 us of SDMA bandwidth from the encode stream at the start.
        y_r = y_in.ap().rearrange("(t p u) -> t p u", p=128, u=G)
        y_tiles = {}

        def load_y(T):
            y_t = spool.tile([128, G], F32, tag="y")
            nc.sync.dma_start(y_t[:], y_r[T])
            y_tiles[T] = y_t

        # first y tiles ahead of the encode stream (128 KiB, ~0.4 us) so the
        # head-time rank chains have their input immediately
        for T in range(4):
            load_y(T)
        load_pair(0, split=True)
        # free-dim iota (elem index within group) for the sort keys
        iota_i = const_pool.tile([128, G], I32)
        nc.gpsimd.iota(iota_i[:], pattern=[[1, G]], base=0, channel_multiplier=0)
        # descending iota (63..0) as int16: data for the rank-producing scatter
        iota_d16 = const_pool.tile([128, G], I16)
        nc.gpsimd.iota(iota_d16[:], pattern=[[-1, G]], base=G - 1,
                       channel_multiplier=0)

        sflat_tiles = {}
        s_tiles = {}
        rank_tiles = {}
        ss_tiles = {}
        th_tiles = {}

        def rank_chain(T):
            """Keys + sort + rank for tile T.  Depends ONLY on y_all (loaded
            at kernel start), so these DVE/gpsimd ops never wait and can be
            emitted arbitrarily early."""
            # keys on DVE: k64 = int(y*2^29 + 2^29) (exact in fp32: y is a
            # multiple of 2^-23, so y*2^29 = 64k and 2^29 + 64k have <= 24
            # significant bits over a 2^6 ulp); keys = k64 | elem_index
            y_t = y_tiles.pop(T)
            k64 = spool.tile([128, G], I32, tag="k64")
            nc.vector.tensor_scalar(out=k64[:], in0=y_t[:],
                                    scalar1=float(1 << 29),
                                    scalar2=float(1 << 29),
                                    op0=Alu.mult, op1=Alu.add)
            keys = spool.tile([128, G], I32, tag="keys")
            nc.vector.tensor_tensor(out=keys[:], in0=k64[:], in1=iota_i[:],
                                    op=Alu.bitwise_or)

            # full descending sort of the int keys on DVE (f32 bitcast views)
            sorted_i = spool.tile([128, G], I32, tag="sorted")
            wka = scr_pool.tile([128, G], I32, tag="wka")
            wkb = scr_pool.tile([128, G], I32, tag="wkb")
            src = keys
            dst_t = wka
            for r in range(8):
                nc.vector.max(sorted_i[:, r * 8:(r + 1) * 8].bitcast(F32),
                              src[:].bitcast(F32))
                if r < 7:
                    nc.vector.match_replace(
                        dst_t[:].bitcast(F32),
                        sorted_i[:, r * 8:(r + 1) * 8].bitcast(F32),
                        src[:].bitcast(F32), 0.0)
                    src, dst_t = dst_t, (wkb if dst_t is wka else wka)

            # perm (descending argsort) = low 6 bits; int16 for the scatter
            perm32 = scr_pool.tile([128, G], I32, tag="perm32")
            nc.vector.tensor_scalar(out=perm32[:], in0=sorted_i[:],
                                    scalar1=63, scalar2=None,
                                    op0=Alu.bitwise_and)
            perm16 = spool.tile([128, G], I16, tag="perm16")
            nc.vector.tensor_copy(perm16[:], perm32[:])
            # rank_asc[i] = position of element i in ascending order
            rank16 = spool.tile([128, G], I16, tag="rank16")
            nc.gpsimd.local_scatter(rank16[:], iota_d16[:], perm16[:],
                                    channels=128, num_elems=G, num_idxs=G)
            rank_tiles[T] = rank16

        def score_scatter(T):
            """Permute tile T's fp16 scores by rank in one gpsimd scatter
            (rank16 and s_t both long since materialized)."""
            s_t = s_tiles.pop(T)
            rank16 = rank_tiles.pop(T)
            ssort = spool.tile([128, G], I16, tag="ssort")
            nc.gpsimd.local_scatter(ssort[:], s_t[:].bitcast(I16), rank16[:],
                                    channels=128, num_elems=G, num_idxs=G)
            ss_tiles[T] = ssort

        def tanh_store(T):
            """negs0 + tanh on Act (ssort long done -> no Act stall)."""
            ssort = ss_tiles.pop(T)
            ssf = ssort[:].bitcast(F16)
            negs0 = spool.tile([128, 1], F32, tag="negs0")
            nc.scalar.mul(negs0[:], ssf[:, 0:1], -1.0)
            th = spool.tile([128, G - 1], F32, tag="th")
            nc.scalar.activation(th[:], ssf[:, 1:G], Act.Tanh,
                                 bias=negs0[:], scale=1.0)
            th_tiles[T] = th

        def store_out(T):
            # lagged far enough that tanh(T) long completed: the sync-ring
            # issue never waits, so it can't block the encode stream
            nc.sync.dma_start(out_r[T], th_tiles.pop(T))

        # rank chains for the first two tiles run during the DMA head while
        # DVE/gpsimd are otherwise idle
        rank_chain(0)
        rank_chain(1)

        for pair in range(PAIRS):
            if pair > 0:
                load_pair(pair)
            etc0, etc1 = etc_tiles.pop(pair)

            for t in range(2):
                T = pair * 2 + t
                # ---- scores for tile T (native-orientation PE matvec) ----
                sflat = sfpool.tile([1, RPT], F16, tag="sflat")
                for j in range(RPT // EXIT_N):
                    ps = ps_pool.tile([1, EXIT_N], F32, tag="ps")
                    for q in range(EXIT_N // MM_N):
                        c0 = t * RPT + j * EXIT_N + q * MM_N
                        nc.tensor.matmul(ps[:, q * MM_N:(q + 1) * MM_N],
                                         wsb_h[:, 0:1], etc0[:, c0:c0 + MM_N],
                                         start=True, stop=False)
                        nc.tensor.matmul(ps[:, q * MM_N:(q + 1) * MM_N],
                                         wsb_h[:, 1:2], etc1[:, c0:c0 + MM_N],
                                         start=False, stop=True)
                    # exit psum -> sbuf fp16, 3:1 Act:DVE.  The DVE exit is
                    # emitted BEFORE this tile's rank chain, so it is never
                    # queued behind one (DVE per-tile work fits the tile
                    # budget).  For the last tile 2:2 halves the drain tail.
                    dst = sflat[:, j * EXIT_N:(j + 1) * EXIT_N]
                    on_dve = (j == 1 or (T == T_TILES - 1 and j == 3))
                    if on_dve:
                        nc.vector.tensor_copy(dst, ps[:])
                    else:
                        nc.scalar.copy(dst, ps[:])

                sflat_tiles[T] = sflat

                # lagged, decoupled tail stages: every op emitted below has
                # its deps already complete when its engine reaches it, so
                # no queue ever head-of-line blocks the psum exits.
                if T >= 1:
                    # relayout flat scores -> [group(partition), elem]; the
                    # sync-ring issue is lagged one tile so it never waits
                    # on the exits (and gpsimd stays scatter-only: mixing
                    # SWDGE DMAs with local_scatter forces a ~2.2 us gpsimd
                    # lib reload per switch)
                    s_t = spool.tile([128, G], F16, tag="s")
                    nc.sync.dma_start(s_t[:], sflat_tiles.pop(T - 1)[:])
                    s_tiles[T - 1] = s_t
                if T + 4 < T_TILES:
                    load_y(T + 4)
                if T + 2 < T_TILES:
                    rank_chain(T + 2)
                if T >= 2:
                    score_scatter(T - 2)
                if T >= 3:
                    tanh_store(T - 3)
                if T >= 4:
                    store_out(T - 4)
        TL = T_TILES - 1
        s_t = spool.tile([128, G], F16, tag="s")
        nc.sync.dma_start(s_t[:], sflat_tiles.pop(TL)[:])
        s_tiles[TL] = s_t
        score_scatter(TL - 1)
        score_scatter(TL)
        for T in (TL - 2, TL - 1, TL):
            tanh_store(T)
        for T in (TL - 3, TL - 2, TL - 1, TL):
            store_out(T)

    nc.compile()
    return nc


last_results = None


def kernel(encodes, y_coord, W, x_coord=None):
    global last_results
    if "nc" not in _built:
        _built["nc"] = _build_nc()
    nc = _built["nc"]

    e16 = np.asarray(encodes).astype(ml_dtypes.bfloat16)
    y_coord = np.ascontiguousarray(np.asarray(y_coord, dtype=np.float32))
    W = np.ascontiguousarray(np.asarray(W, dtype=np.float32))

    in_maps = []
    for c in range(N_CORES):
        # [ROWS, 256] -> [256, ROWS] -> [2, 128, 8, 2, 8192] (c d pair t n)
        # -> [2, 8, 128, 2, 8192] (c pair d t n), each [pair] slice one
        # dense 4 MiB extent per d-chunk
        et_c = np.ascontiguousarray(
            e16[c * ROWS:(c + 1) * ROWS].T
            .reshape(2, 128, PAIRS, 2, RPT)
            .transpose(0, 2, 1, 3, 4))
        in_maps.append({
            "et": et_c,
            "y_coord": y_coord[c * ROWS:(c + 1) * ROWS],
            "w": W,
        })
    # Only request tracing when the axon NTFF hook is importable; otherwise
    # force it off (bass_utils would crash importing antenv.axon_hooks if
    # BASS_TRACE leaked into the environment without the shim installed).
    want_trace = bool(os.environ.get("BASS_TRACE"))
    if want_trace:
        try:
            from antenv.axon_hooks import get_axon_ntff_profile_hook  # noqa: F401
        except ImportError:
            want_trace = False
            os.environ["BASS_NEVER_TRACE"] = "1"
    res = run_bass_kernel_spmd(
        nc, in_maps, core_ids=list(range(N_CORES)),
        trace=want_trace,
    )
    last_results = res
    result = np.concatenate([r["result"] for r in res.results])
    polarity = np.ones(NG * (G - 1), dtype=np.float32)
    return result, polarity


# revision 39
# speedup vs baseline: 1.4462x; 1.0136x over previous
# Trainium2 Bass kernel for nn_DirectRanker (ragged_sequence).
#
# Math shortcut: result = tanh((sorted_enc[:,1:,:] - sorted_enc[:,:1,:]) @ W.T)
# commutes with the linear map, so per-row scores s = encodes @ W.T are
# computed FIRST (the memory-bound part: 512 MiB of bf16 streamed once), and
# the per-group sort/diff/tanh runs on the tiny [N] score vector:
#   result[g, k-1] = tanh(s_sorted[g, k] - s_sorted[g, 0]),  k = 1..63
#
# Sharding: groups split across 8 cores (2048 groups/core), no cross-core
# communication.
#
# Layout: encodes is pre-arranged ON HOST to [16, 128, 2, 8192] bf16
# (tile, d, d-chunk, col) so every encode DMA is ONE dense 4 MiB linear HBM
# extent (partition stride 32 KiB): linear extents measure 383 GB/s/core vs
# 308 for strided.  TensorE computes the matvec in its native orientation:
#   psum[1, 512] += W_chunk[128, 1].T @ ET_chunk[128, 512]
# (2 cycles/row, no on-chip transposes; matmuls stream back-to-back at
# ~215 ns).  Each [1, 2048] psum chunk exits to SBUF fp16 via TWO concurrent
# copies (Act cols 0:1536, DVE cols 1536:2048) so the psum slot is freed
# ~1.9 us after its matmuls and neither engine is the pole.  A single
# SBUF->SBUF DMA per tile relayouts the flat scores into
# [group(partition), elem(free)] for the sort.
#
# Queue discipline (each engine queue is strict FIFO, so an op with an
# unresolved dep head-of-line blocks everything behind it):
#  - sync HWDGE ring: encode stream + all small lagged DMAs (y lookahead,
#    score relayout lag-1, result store lag-4) -- the lagged ops' deps are
#    complete when emitted, so they issue instantly and never stall the ring.
#  - scalar ring: just the tiny W load at t=0.
#  - gpsimd: iotas + local_scatters ONLY (mixing SWDGE DMA issues with
#    scatters forces a ~2.2 us gpsimd lib reload per switch).
#  - DVE: per-chunk half-exits first, then rank chains (sort of y-keys,
#    which depend only on the early y tiles -- never stall).
#  - Act: per-chunk main exits + (lag-3) negs0/tanh.
# y is loaded per tile (128 descriptors) with 4-tile lookahead: one big
# upfront y DMA (2048 tiny descriptors) steals ~20 us of SDMA bandwidth
# from the encode stream at the start.
#
# Exact stable argsort over y within each 64-row group: integer keys
#   key = int(y * 2^29 + 2^29) | elem_index   (exact: y is a multiple of
# 2^-23) are sorted through their f32 bitcast views (monotone for positive
# int32) with 8 rounds of DVE max8 + match_replace; perm = low 6 bits.  The
# score permutation runs on gpsimd local_scatter (fp16 as int16), ranks
# from scattering a descending iota by perm.
import os
from contextlib import ExitStack

import ml_dtypes
import numpy as np

import concourse.bacc as bacc
import concourse.mybir as mybir
import concourse.tile as tile
from concourse.bass_utils import run_bass_kernel_spmd

N_CORES = 8
N = 1048576
D = 256
G = 64
NG = N // G                # 16384 groups
ROWS = N // N_CORES        # 131072 rows per core
GPC = NG // N_CORES        # 2048 groups per core
T_TILES = GPC // 128       # 16 tiles of 128 groups (8192 rows) per core
RPT = 128 * G              # rows per tile = 8192
MM_N = 512                 # moving free size per matmul (1 psum bank)
EXIT_N = 2048              # scores per psum chunk (4 psum banks)
EXIT_SPLIT = 1536          # Act exits [0:1536], DVE exits [1536:2048]
F32 = mybir.dt.float32
F16 = mybir.dt.float16
BF16 = mybir.dt.bfloat16
I32 = mybir.dt.int32
I16 = mybir.dt.int16
Alu = mybir.AluOpType
Act = mybir.ActivationFunctionType

_built = {}


def _build_nc():
    nc = bacc.Bacc("TRN2", target_bir_lowering=False, debug=False,
                   num_devices=N_CORES)
    # host-prearranged encodes: [tile, d, d-chunk, col]
    et_in = nc.dram_tensor("et", [T_TILES, 128, 2, RPT], BF16,
                           kind="ExternalInput")
    y_in = nc.dram_tensor("y_coord", [ROWS], F32, kind="ExternalInput")
    w_in = nc.dram_tensor("w", [1, D], F32, kind="ExternalInput")
    out = nc.dram_tensor("result", [GPC * (G - 1)], F32, kind="ExternalOutput")

    out_r = out.ap().rearrange("(t p k) -> t p k", p=128, k=G - 1)

    with tile.TileContext(nc) as tc, ExitStack() as ctx:
        const_pool = ctx.enter_context(tc.tile_pool(name="const", bufs=1))
        epool = ctx.enter_context(tc.tile_pool(name="e", bufs=4))
        sfpool = ctx.enter_context(tc.tile_pool(name="sf", bufs=3))
        spool = ctx.enter_context(tc.tile_pool(name="s", bufs=6))
        scr_pool = ctx.enter_context(tc.tile_pool(name="scr", bufs=3))
        ps_pool = ctx.enter_context(
            tc.tile_pool(name="ps", bufs=2, space="PSUM"))

        # W on the idle scalar ring so it lands in ~2 us (on the sync ring
        # it would queue behind MiBs of encodes)
        wsb = const_pool.tile([128, 2], F32)
        nc.scalar.dma_start(wsb[:],
                            w_in.ap()[0, :].rearrange("(c p) -> p c", p=128))
        wsb_h = const_pool.tile([128, 2], BF16)
        nc.vector.tensor_copy(wsb_h[:], wsb[:])

        y_r = y_in.ap().rearrange("(t p u) -> t p u", p=128, u=G)
        y_tiles = {}

        def load_y(T):
            y_t = spool.tile([128, G], F32, tag="y")
            nc.sync.dma_start(y_t[:], y_r[T])
            y_tiles[T] = y_t

        ett_tiles = {}

        def load_tile(T, split=False):
            ett = epool.tile([128, 2 * RPT], BF16, tag="ett")
            src = et_in.ap()[T].rearrange("p c n -> p (c n)")
            if split:
                # two half-col extents so the first chunks' matmuls start
                # ~6 us earlier (pipeline head only)
                h = RPT // 2
                nc.sync.dma_start(
                    ett[:].rearrange("p (c n) -> p c n", c=2)[:, :, 0:h],
                    et_in.ap()[T][:, :, 0:h])
                nc.sync.dma_start(
                    ett[:].rearrange("p (c n) -> p c n", c=2)[:, :, h:RPT],
                    et_in.ap()[T][:, :, h:RPT])
            else:
                nc.sync.dma_start(ett[:], src)
            ett_tiles[T] = ett

        # first y tiles ahead of the encode stream (128 KiB, ~0.4 us)
        for T in range(4):
            load_y(T)
        load_tile(0, split=True)
        load_tile(1)

        # free-dim iota (elem index within group) for the sort keys
        iota_i = const_pool.tile([128, G], I32)
        nc.gpsimd.iota(iota_i[:], pattern=[[1, G]], base=0, channel_multiplier=0)
        # descending iota (63..0) as int16: data for the rank-producing scatter
        iota_d16 = const_pool.tile([128, G], I16)
        nc.gpsimd.iota(iota_d16[:], pattern=[[-1, G]], base=G - 1,
                       channel_multiplier=0)

        sflat_tiles = {}
        s_tiles = {}
        rank_tiles = {}
        ss_tiles = {}
        th_tiles = {}

        def rank_chain(T):
            """Keys + sort + rank for tile T (deps: y tile loaded 4 tiles
            ahead + iotas; never waits at execution time)."""
            y_t = y_tiles.pop(T)
            # keys on DVE: k64 = int(y*2^29 + 2^29) (exact in fp32: y is a
            # multiple of 2^-23), keys = k64 | elem_index
            k64 = spool.tile([128, G], I32, tag="k64")
            nc.vector.tensor_scalar(out=k64[:], in0=y_t[:],
                                    scalar1=float(1 << 29),
                                    scalar2=float(1 << 29),
                                    op0=Alu.mult, op1=Alu.add)
            keys = spool.tile([128, G], I32, tag="keys")
            nc.vector.tensor_tensor(out=keys[:], in0=k64[:], in1=iota_i[:],
                                    op=Alu.bitwise_or)

            # full descending sort of the int keys on DVE (f32 bitcast views
            # order identically to the positive int32 keys)
            sorted_i = spool.tile([128, G], I32, tag="sorted")
            wka = scr_pool.tile([128, G], I32, tag="wka")
            wkb = scr_pool.tile([128, G], I32, tag="wkb")
            src = keys
            dst_t = wka
            for r in range(8):
                nc.vector.max(sorted_i[:, r * 8:(r + 1) * 8].bitcast(F32),
                              src[:].bitcast(F32))
                if r < 7:
                    nc.vector.match_replace(
                        dst_t[:].bitcast(F32),
                        sorted_i[:, r * 8:(r + 1) * 8].bitcast(F32),
                        src[:].bitcast(F32), 0.0)
                    src, dst_t = dst_t, (wkb if dst_t is wka else wka)

            # perm (descending argsort) = low 6 bits; int16 for the scatter
            perm32 = scr_pool.tile([128, G], I32, tag="perm32")
            nc.vector.tensor_scalar(out=perm32[:], in0=sorted_i[:],
                                    scalar1=63, scalar2=None,
                                    op0=Alu.bitwise_and)
            perm16 = spool.tile([128, G], I16, tag="perm16")
            nc.vector.tensor_copy(perm16[:], perm32[:])
            # rank_asc[i] = position of element i in ascending order
            rank16 = spool.tile([128, G], I16, tag="rank16")
            nc.gpsimd.local_scatter(rank16[:], iota_d16[:], perm16[:],
                                    channels=128, num_elems=G, num_idxs=G)
            rank_tiles[T] = rank16

        def score_scatter(T):
            s_t = s_tiles.pop(T)
            rank16 = rank_tiles.pop(T)
            ssort = spool.tile([128, G], I16, tag="ssort")
            nc.gpsimd.local_scatter(ssort[:], s_t[:].bitcast(I16), rank16[:],
                                    channels=128, num_elems=G, num_idxs=G)
            ss_tiles[T] = ssort

        def tanh_store(T):
            ssort = ss_tiles.pop(T)
            ssf = ssort[:].bitcast(F16)
            negs0 = spool.tile([128, 1], F32, tag="negs0")
            nc.scalar.mul(negs0[:], ssf[:, 0:1], -1.0)
            th = spool.tile([128, G - 1], F32, tag="th")
            nc.scalar.activation(th[:], ssf[:, 1:G], Act.Tanh,
                                 bias=negs0[:], scale=1.0)
            th_tiles[T] = th

        def store_out(T):
            nc.sync.dma_start(out_r[T], th_tiles.pop(T))

        def relayout(T):
            s_t = spool.tile([128, G], F16, tag="s")
            nc.sync.dma_start(s_t[:], sflat_tiles.pop(T)[:])
            s_tiles[T] = s_t

        # rank chains for the first two tiles run during the DMA head
        rank_chain(0)
        rank_chain(1)

        for T in range(T_TILES):
            if T + 2 < T_TILES:
                load_tile(T + 2)
            ett = ett_tiles.pop(T)

            # ---- scores for tile T (native-orientation PE matvec) ----
            sflat = sfpool.tile([1, RPT], F16, tag="sflat")
            for j in range(RPT // EXIT_N):
                ps = ps_pool.tile([1, EXIT_N], F32, tag="ps")
                for q in range(EXIT_N // MM_N):
                    c0 = j * EXIT_N + q * MM_N
                    nc.tensor.matmul(ps[:, q * MM_N:(q + 1) * MM_N],
                                     wsb_h[:, 0:1], ett[:, c0:c0 + MM_N],
                                     start=True, stop=False)
                    nc.tensor.matmul(ps[:, q * MM_N:(q + 1) * MM_N],
                                     wsb_h[:, 1:2],
                                     ett[:, RPT + c0:RPT + c0 + MM_N],
                                     start=False, stop=True)
                # split exit: Act + DVE halves run concurrently
                base = j * EXIT_N
                nc.scalar.copy(sflat[:, base:base + EXIT_SPLIT],
                               ps[:, 0:EXIT_SPLIT])
                nc.vector.tensor_copy(sflat[:, base + EXIT_SPLIT:base + EXIT_N],
                                      ps[:, EXIT_SPLIT:EXIT_N])

            sflat_tiles[T] = sflat

            # lagged, decoupled tail stages (deps complete when emitted)
            if T >= 1:
                relayout(T - 1)
            if T + 4 < T_TILES:
                load_y(T + 4)
            if T + 2 < T_TILES:
                rank_chain(T + 2)
            if T >= 2:
                score_scatter(T - 2)
            if T >= 3:
                tanh_store(T - 3)
            if T >= 4:
                store_out(T - 4)

        TL = T_TILES - 1
        relayout(TL)
        score_scatter(TL - 1)
        score_scatter(TL)
        for T in (TL - 2, TL - 1, TL):
            tanh_store(T)
        for T in (TL - 3, TL - 2, TL - 1, TL):
            store_out(T)

    nc.compile()
    return nc


last_results = None


def kernel(encodes, y_coord, W, x_coord=None):
    global last_results
    if "nc" not in _built:
        _built["nc"] = _build_nc()
    nc = _built["nc"]

    e16 = np.asarray(encodes).astype(ml_dtypes.bfloat16)
    y_coord = np.ascontiguousarray(np.asarray(y_coord, dtype=np.float32))
    W = np.ascontiguousarray(np.asarray(W, dtype=np.float32))

    in_maps = []
    for c in range(N_CORES):
        # [ROWS, 256] -> [256, ROWS] -> [2, 128, 16, 8192] (c d T n)
        # -> [16, 128, 2, 8192] (T d c n): each [T] slice one dense 4 MiB
        # extent with partition (d) stride 32 KiB
        et_c = np.ascontiguousarray(
            e16[c * ROWS:(c + 1) * ROWS].T
            .reshape(2, 128, T_TILES, RPT)
            .transpose(2, 1, 0, 3))
        in_maps.append({
            "et": et_c,
            "y_coord": y_coord[c * ROWS:(c + 1) * ROWS],
            "w": W,
        })
    # Only request tracing when the axon NTFF hook is importable; otherwise
    # force it off (bass_utils would crash importing antenv.axon_hooks if
    # BASS_TRACE leaked into the environment without the shim installed).
    want_trace = bool(os.environ.get("BASS_TRACE"))
    if want_trace:
        try:
            from antenv.axon_hooks import get_axon_ntff_profile_hook  # noqa: F401
        except ImportError:
            want_trace = False
            os.environ["BASS_NEVER_TRACE"] = "1"
    res = run_bass_kernel_spmd(
        nc, in_maps, core_ids=list(range(N_CORES)),
        trace=want_trace,
    )
    last_results = res
    result = np.concatenate([r["result"] for r in res.results])
    polarity = np.ones(NG * (G - 1), dtype=np.float32)
    return result, polarity


# revision 40
# speedup vs baseline: 1.4467x; 1.0003x over previous
# Trainium2 Bass kernel for nn_DirectRanker (ragged_sequence).
#
# Math shortcut: result = tanh((sorted_enc[:,1:,:] - sorted_enc[:,:1,:]) @ W.T)
# commutes with the linear map, so per-row scores s = encodes @ W.T are
# computed FIRST (the memory-bound part: 512 MiB of bf16 streamed once), and
# the per-group sort/diff/tanh runs on the tiny [N] score vector:
#   result[g, k-1] = tanh(s_sorted[g, k] - s_sorted[g, 0]),  k = 1..63
#
# Sharding: groups split across 8 cores (2048 groups/core), no cross-core
# communication.
#
# Layout: encodes is pre-arranged ON HOST to [16, 128, 2, 8192] bf16
# (tile, d, d-chunk, col) so every encode DMA is ONE dense 4 MiB linear HBM
# extent (partition stride 32 KiB): linear extents measure 383 GB/s/core vs
# 308 for strided.  TensorE computes the matvec in its native orientation:
#   psum[1, 512] += W_chunk[128, 1].T @ ET_chunk[128, 512]
# (2 cycles/row, no on-chip transposes; matmuls stream back-to-back at
# ~215 ns).  Each [1, 2048] psum chunk exits to SBUF fp16 via TWO concurrent
# copies (Act cols 0:1536, DVE cols 1536:2048) so the psum slot is freed
# ~1.9 us after its matmuls and neither engine is the pole.  A single
# SBUF->SBUF DMA per tile relayouts the flat scores into
# [group(partition), elem(free)] for the sort.
#
# Queue discipline (each engine queue is strict FIFO, so an op with an
# unresolved dep head-of-line blocks everything behind it):
#  - sync HWDGE ring: encode stream + all small lagged DMAs (y lookahead,
#    score relayout lag-1, result store lag-4) -- the lagged ops' deps are
#    complete when emitted, so they issue instantly and never stall the ring.
#  - scalar ring: just the tiny W load at t=0.
#  - gpsimd: iotas + local_scatters ONLY (mixing SWDGE DMA issues with
#    scatters forces a ~2.2 us gpsimd lib reload per switch).
#  - DVE: per-chunk half-exits first, then rank chains (sort of y-keys,
#    which depend only on the early y tiles -- never stall).
#  - Act: per-chunk main exits + (lag-3) negs0/tanh.
# y is loaded per tile (128 descriptors) with 4-tile lookahead: one big
# upfront y DMA (2048 tiny descriptors) steals ~20 us of SDMA bandwidth
# from the encode stream at the start.
#
# Exact stable argsort over y within each 64-row group: integer keys
#   key = int(y * 2^29 + 2^29) | elem_index   (exact: y is a multiple of
# 2^-23) are sorted through their f32 bitcast views (monotone for positive
# int32) with 8 rounds of DVE max8 + match_replace; perm = low 6 bits.  The
# score permutation runs on gpsimd local_scatter (fp16 as int16), ranks
# from scattering a descending iota by perm.
import os
from contextlib import ExitStack

import ml_dtypes
import numpy as np

import concourse.bacc as bacc
import concourse.mybir as mybir
import concourse.tile as tile
from concourse.bass_utils import run_bass_kernel_spmd

N_CORES = 8
N = 1048576
D = 256
G = 64
NG = N // G                # 16384 groups
ROWS = N // N_CORES        # 131072 rows per core
GPC = NG // N_CORES        # 2048 groups per core
T_TILES = GPC // 128       # 16 tiles of 128 groups (8192 rows) per core
RPT = 128 * G              # rows per tile = 8192
MM_N = 512                 # moving free size per matmul (1 psum bank)
EXIT_N = 2048              # scores per psum chunk (4 psum banks)
EXIT_SPLIT = 1536          # Act exits [0:1536], DVE exits [1536:2048]
F32 = mybir.dt.float32
F16 = mybir.dt.float16
BF16 = mybir.dt.bfloat16
I32 = mybir.dt.int32
I16 = mybir.dt.int16
Alu = mybir.AluOpType
Act = mybir.ActivationFunctionType

_built = {}


def _build_nc():
    nc = bacc.Bacc("TRN2", target_bir_lowering=False, debug=False,
                   num_devices=N_CORES)
    # host-prearranged encodes: [tile, d, d-chunk, col]
    et_in = nc.dram_tensor("et", [T_TILES, 128, 2, RPT], BF16,
                           kind="ExternalInput")
    y_in = nc.dram_tensor("y_coord", [ROWS], F32, kind="ExternalInput")
    w_in = nc.dram_tensor("w", [1, D], F32, kind="ExternalInput")
    out = nc.dram_tensor("result", [GPC * (G - 1)], F32, kind="ExternalOutput")

    out_r = out.ap().rearrange("(t p k) -> t p k", p=128, k=G - 1)

    with tile.TileContext(nc) as tc, ExitStack() as ctx:
        const_pool = ctx.enter_context(tc.tile_pool(name="const", bufs=1))
        epool = ctx.enter_context(tc.tile_pool(name="e", bufs=4))
        sfpool = ctx.enter_context(tc.tile_pool(name="sf", bufs=3))
        spool = ctx.enter_context(tc.tile_pool(name="s", bufs=6))
        scr_pool = ctx.enter_context(tc.tile_pool(name="scr", bufs=3))
        ps_pool = ctx.enter_context(
            tc.tile_pool(name="ps", bufs=2, space="PSUM"))

        # W on the idle scalar ring so it lands in ~2 us (on the sync ring
        # it would queue behind MiBs of encodes)
        wsb = const_pool.tile([128, 2], F32)
        nc.scalar.dma_start(wsb[:],
                            w_in.ap()[0, :].rearrange("(c p) -> p c", p=128))
        wsb_h = const_pool.tile([128, 2], BF16)
        nc.vector.tensor_copy(wsb_h[:], wsb[:])

        y_r = y_in.ap().rearrange("(t p u) -> t p u", p=128, u=G)
        y_tiles = {}

        def load_y(T):
            y_t = spool.tile([128, G], F32, tag="y")
            nc.sync.dma_start(y_t[:], y_r[T])
            y_tiles[T] = y_t

        ett_tiles = {}

        def load_tile(T, split=False):
            ett = epool.tile([128, 2 * RPT], BF16, tag="ett")
            src = et_in.ap()[T].rearrange("p c n -> p (c n)")
            if split:
                # two half-col extents so the first chunks' matmuls start
                # ~6 us earlier (pipeline head only)
                h = RPT // 2
                nc.sync.dma_start(
                    ett[:].rearrange("p (c n) -> p c n", c=2)[:, :, 0:h],
                    et_in.ap()[T][:, :, 0:h])
                nc.sync.dma_start(
                    ett[:].rearrange("p (c n) -> p c n", c=2)[:, :, h:RPT],
                    et_in.ap()[T][:, :, h:RPT])
            else:
                nc.sync.dma_start(ett[:], src)
            ett_tiles[T] = ett

        # first y tiles ahead of the encode stream (128 KiB, ~0.4 us)
        for T in range(4):
            load_y(T)
        load_tile(0, split=True)
        load_tile(1)

        # free-dim iota (elem index within group) for the sort keys
        iota_i = const_pool.tile([128, G], I32)
        nc.gpsimd.iota(iota_i[:], pattern=[[1, G]], base=0, channel_multiplier=0)
        # descending iota (63..0) as int16: data for the rank-producing scatter
        iota_d16 = const_pool.tile([128, G], I16)
        nc.gpsimd.iota(iota_d16[:], pattern=[[-1, G]], base=G - 1,
                       channel_multiplier=0)

        sflat_tiles = {}
        s_tiles = {}
        rank_tiles = {}
        ss_tiles = {}
        th_tiles = {}

        def rank_chain(T):
            """Keys + sort + rank for tile T (deps: y tile loaded 4 tiles
            ahead + iotas; never waits at execution time)."""
            y_t = y_tiles.pop(T)
            # keys on DVE: k64 = int(y*2^29 + 2^29) (exact in fp32: y is a
            # multiple of 2^-23), keys = k64 | elem_index
            k64 = spool.tile([128, G], I32, tag="k64")
            nc.vector.tensor_scalar(out=k64[:], in0=y_t[:],
                                    scalar1=float(1 << 29),
                                    scalar2=float(1 << 29),
                                    op0=Alu.mult, op1=Alu.add)
            keys = spool.tile([128, G], I32, tag="keys")
            nc.vector.tensor_tensor(out=keys[:], in0=k64[:], in1=iota_i[:],
                                    op=Alu.bitwise_or)

            # full descending sort of the int keys on DVE (f32 bitcast views
            # order identically to the positive int32 keys)
            sorted_i = spool.tile([128, G], I32, tag="sorted")
            wka = scr_pool.tile([128, G], I32, tag="wka")
            wkb = scr_pool.tile([128, G], I32, tag="wkb")
            src = keys
            dst_t = wka
            for r in range(8):
                nc.vector.max(sorted_i[:, r * 8:(r + 1) * 8].bitcast(F32),
                              src[:].bitcast(F32))
                if r < 7:
                    nc.vector.match_replace(
                        dst_t[:].bitcast(F32),
                        sorted_i[:, r * 8:(r + 1) * 8].bitcast(F32),
                        src[:].bitcast(F32), 0.0)
                    src, dst_t = dst_t, (wkb if dst_t is wka else wka)

            # perm (descending argsort) = low 6 bits; int16 for the scatter
            perm32 = scr_pool.tile([128, G], I32, tag="perm32")
            nc.vector.tensor_scalar(out=perm32[:], in0=sorted_i[:],
                                    scalar1=63, scalar2=None,
                                    op0=Alu.bitwise_and)
            perm16 = spool.tile([128, G], I16, tag="perm16")
            nc.vector.tensor_copy(perm16[:], perm32[:])
            # rank_asc[i] = position of element i in ascending order
            rank16 = spool.tile([128, G], I16, tag="rank16")
            nc.gpsimd.local_scatter(rank16[:], iota_d16[:], perm16[:],
                                    channels=128, num_elems=G, num_idxs=G)
            rank_tiles[T] = rank16

        def score_scatter(T):
            s_t = s_tiles.pop(T)
            rank16 = rank_tiles.pop(T)
            ssort = spool.tile([128, G], I16, tag="ssort")
            nc.gpsimd.local_scatter(ssort[:], s_t[:].bitcast(I16), rank16[:],
                                    channels=128, num_elems=G, num_idxs=G)
            ss_tiles[T] = ssort

        def tanh_store(T):
            ssort = ss_tiles.pop(T)
            ssf = ssort[:].bitcast(F16)
            negs0 = spool.tile([128, 1], F32, tag="negs0")
            nc.scalar.mul(negs0[:], ssf[:, 0:1], -1.0)
            th = spool.tile([128, G - 1], F32, tag="th")
            nc.scalar.activation(th[:], ssf[:, 1:G], Act.Tanh,
                                 bias=negs0[:], scale=1.0)
            th_tiles[T] = th

        def store_out(T):
            nc.sync.dma_start(out_r[T], th_tiles.pop(T))

        def relayout(T):
            s_t = spool.tile([128, G], F16, tag="s")
            nc.sync.dma_start(s_t[:], sflat_tiles.pop(T)[:])
            s_tiles[T] = s_t

        # rank chains for the first two tiles run during the DMA head
        rank_chain(0)
        rank_chain(1)

        for T in range(T_TILES):
            if T + 2 < T_TILES:
                load_tile(T + 2)
            ett = ett_tiles.pop(T)

            # ---- scores for tile T (native-orientation PE matvec) ----
            sflat = sfpool.tile([1, RPT], F16, tag="sflat")
            for j in range(RPT // EXIT_N):
                ps = ps_pool.tile([1, EXIT_N], F32, tag="ps")
                for q in range(EXIT_N // MM_N):
                    c0 = j * EXIT_N + q * MM_N
                    nc.tensor.matmul(ps[:, q * MM_N:(q + 1) * MM_N],
                                     wsb_h[:, 0:1], ett[:, c0:c0 + MM_N],
                                     start=True, stop=False)
                    nc.tensor.matmul(ps[:, q * MM_N:(q + 1) * MM_N],
                                     wsb_h[:, 1:2],
                                     ett[:, RPT + c0:RPT + c0 + MM_N],
                                     start=False, stop=True)
                # split exit: Act + DVE halves run concurrently
                base = j * EXIT_N
                nc.scalar.copy(sflat[:, base:base + EXIT_SPLIT],
                               ps[:, 0:EXIT_SPLIT])
                nc.vector.tensor_copy(sflat[:, base + EXIT_SPLIT:base + EXIT_N],
                                      ps[:, EXIT_SPLIT:EXIT_N])

            sflat_tiles[T] = sflat

            # lagged, decoupled tail stages.  relayout at lag 2: at lag 1
            # its dep (the last exit of T-1) completes a hair after emission
            # and the wait at the sync-ring head delays every later encode
            # transfer by ~1.5 us/tile.
            if T >= 2:
                relayout(T - 2)
            if T + 4 < T_TILES:
                load_y(T + 4)
            if T + 2 < T_TILES:
                rank_chain(T + 2)
            if T >= 3:
                score_scatter(T - 3)
            if T >= 4:
                tanh_store(T - 4)
            if T >= 5:
                store_out(T - 5)

        TL = T_TILES - 1
        for T in (TL - 1, TL):
            relayout(T)
        for T in (TL - 2, TL - 1, TL):
            score_scatter(T)
        for T in (TL - 3, TL - 2, TL - 1, TL):
            tanh_store(T)
        for T in (TL - 4, TL - 3, TL - 2, TL - 1, TL):
            store_out(T)

    nc.compile()
    return nc


last_results = None


def kernel(encodes, y_coord, W, x_coord=None):
    global last_results
    if "nc" not in _built:
        _built["nc"] = _build_nc()
    nc = _built["nc"]

    e16 = np.asarray(encodes).astype(ml_dtypes.bfloat16)
    y_coord = np.ascontiguousarray(np.asarray(y_coord, dtype=np.float32))
    W = np.ascontiguousarray(np.asarray(W, dtype=np.float32))

    in_maps = []
    for c in range(N_CORES):
        # [ROWS, 256] -> [256, ROWS] -> [2, 128, 16, 8192] (c d T n)
        # -> [16, 128, 2, 8192] (T d c n): each [T] slice one dense 4 MiB
        # extent with partition (d) stride 32 KiB
        et_c = np.ascontiguousarray(
            e16[c * ROWS:(c + 1) * ROWS].T
            .reshape(2, 128, T_TILES, RPT)
            .transpose(2, 1, 0, 3))
        in_maps.append({
            "et": et_c,
            "y_coord": y_coord[c * ROWS:(c + 1) * ROWS],
            "w": W,
        })
    # Only request tracing when the axon NTFF hook is importable; otherwise
    # force it off (bass_utils would crash importing antenv.axon_hooks if
    # BASS_TRACE leaked into the environment without the shim installed).
    want_trace = bool(os.environ.get("BASS_TRACE"))
    if want_trace:
        try:
            from antenv.axon_hooks import get_axon_ntff_profile_hook  # noqa: F401
        except ImportError:
            want_trace = False
            os.environ["BASS_NEVER_TRACE"] = "1"
    res = run_bass_kernel_spmd(
        nc, in_maps, core_ids=list(range(N_CORES)),
        trace=want_trace,
    )
    last_results = res
    result = np.concatenate([r["result"] for r in res.results])
    polarity = np.ones(NG * (G - 1), dtype=np.float32)
    return result, polarity


# revision 46
# speedup vs baseline: 1.8032x; 1.2464x over previous
# Trainium2 Bass kernel for nn_DirectRanker (ragged_sequence).
#
# Math shortcut: result = tanh((sorted_enc[:,1:,:] - sorted_enc[:,:1,:]) @ W.T)
# commutes with the linear map, so per-row scores s = encodes @ W.T are
# computed FIRST (the memory-bound part: 512 MiB of bf16 streamed once), and
# the per-group sort/diff/tanh runs on the tiny [N] score vector:
#   result[g, k-1] = tanh(s_sorted[g, k] - s_sorted[g, 0]),  k = 1..63
#
# Sharding: groups split across 8 cores (2048 groups/core), no cross-core
# communication.
#
# Layout: encodes is pre-arranged ON HOST to [16, 128, 2, 8192] bf16
# (tile, d, d-chunk, col) so every encode DMA is ONE dense 4 MiB linear HBM
# extent (partition stride 32 KiB): linear extents measure 383 GB/s/core vs
# 308 for strided.  TensorE computes the matvec in its native orientation:
#   psum[1, 512] += W_chunk[128, 1].T @ ET_chunk[128, 512]
# (2 cycles/row, no on-chip transposes; matmuls stream back-to-back at
# ~215 ns).  Each [1, 2048] psum chunk exits to SBUF fp16 via TWO concurrent
# copies (Act cols 0:1536, DVE cols 1536:2048) so the psum slot is freed
# ~1.9 us after its matmuls and neither engine is the pole.  A single
# SBUF->SBUF DMA per tile relayouts the flat scores into
# [group(partition), elem(free)] for the sort.
#
# Queue discipline (each engine queue is strict FIFO, so an op with an
# unresolved dep head-of-line blocks everything behind it):
#  - sync HWDGE ring: encode stream + all small lagged DMAs (y lookahead,
#    score relayout lag-1, result store lag-4) -- the lagged ops' deps are
#    complete when emitted, so they issue instantly and never stall the ring.
#  - scalar ring: just the tiny W load at t=0.
#  - gpsimd: iotas + local_scatters ONLY (mixing SWDGE DMA issues with
#    scatters forces a ~2.2 us gpsimd lib reload per switch).
#  - DVE: per-chunk half-exits first, then rank chains (sort of y-keys,
#    which depend only on the early y tiles -- never stall).
#  - Act: per-chunk main exits + (lag-3) negs0/tanh.
# y is loaded per tile (128 descriptors) with 4-tile lookahead: one big
# upfront y DMA (2048 tiny descriptors) steals ~20 us of SDMA bandwidth
# from the encode stream at the start.
#
# Exact stable argsort over y within each 64-row group: integer keys
#   key = int(y * 2^29 + 2^29) | elem_index   (exact: y is a multiple of
# 2^-23) are sorted through their f32 bitcast views (monotone for positive
# int32) with 8 rounds of DVE max8 + match_replace; perm = low 6 bits.  The
# score permutation runs on gpsimd local_scatter (fp16 as int16), ranks
# from scattering a descending iota by perm.
import os
from contextlib import ExitStack

import ml_dtypes
import numpy as np

import concourse.bacc as bacc
import concourse.mybir as mybir
import concourse.tile as tile
from concourse.bass_utils import run_bass_kernel_spmd

N_CORES = 8
N = 1048576
D = 256
G = 64
NG = N // G                # 16384 groups
ROWS = N // N_CORES        # 131072 rows per core
GPC = NG // N_CORES        # 2048 groups per core
T_TILES = GPC // 128       # 16 tiles of 128 groups (8192 rows) per core
RPT = 128 * G              # rows per tile = 8192
MM_N = 512                 # moving free size per matmul (1 psum bank)
EXIT_N = 2048              # scores per psum chunk (4 psum banks)
EXIT_SPLIT = 1536          # Act exits [0:1536], DVE exits [1536:2048]
F32 = mybir.dt.float32
F16 = mybir.dt.float16
BF16 = mybir.dt.bfloat16
F8 = mybir.dt.float8e4
I32 = mybir.dt.int32
I16 = mybir.dt.int16
Alu = mybir.AluOpType
Act = mybir.ActivationFunctionType

_built = {}


def _build_nc():
    nc = bacc.Bacc("TRN2", target_bir_lowering=False, debug=False,
                   num_devices=N_CORES)
    # host-prearranged encodes, dims permuted by descending |W| and split:
    # the 128 large-|W| dims in bf16, the 128 small-|W| dims in fp8-e4m3
    # (they carry ~13% of the variance-weighted quantization error; rel err
    # measured 0.98e-2 vs the 2e-2 gate).  [tile, d, col] each, every tile
    # slice one dense linear HBM extent (2 MiB / 1 MiB).
    et16_in = nc.dram_tensor("et16", [T_TILES, 128, RPT], BF16,
                             kind="ExternalInput")
    et8_in = nc.dram_tensor("et8", [T_TILES, 128, RPT], F8,
                            kind="ExternalInput")
    y_in = nc.dram_tensor("y_coord", [ROWS], F32, kind="ExternalInput")
    w_in = nc.dram_tensor("w", [1, D], F32, kind="ExternalInput")
    out = nc.dram_tensor("result", [GPC * (G - 1)], F32, kind="ExternalOutput")

    out_r = out.ap().rearrange("(t p k) -> t p k", p=128, k=G - 1)

    with tile.TileContext(nc) as tc, ExitStack() as ctx:
        const_pool = ctx.enter_context(tc.tile_pool(name="const", bufs=1))
        epool = ctx.enter_context(tc.tile_pool(name="e", bufs=4))
        sfpool = ctx.enter_context(tc.tile_pool(name="sf", bufs=3))
        spool = ctx.enter_context(tc.tile_pool(name="s", bufs=6))
        scr_pool = ctx.enter_context(tc.tile_pool(name="scr", bufs=3))
        ps_pool = ctx.enter_context(
            tc.tile_pool(name="ps", bufs=2, space="PSUM"))

        # W on the idle scalar ring so it lands in ~2 us (on the sync ring
        # it would queue behind MiBs of encodes)
        wsb = const_pool.tile([128, 2], F32)
        nc.scalar.dma_start(wsb[:],
                            w_in.ap()[0, :].rearrange("(c p) -> p c", p=128))
        wsb_h = const_pool.tile([128, 2], BF16)
        nc.vector.tensor_copy(wsb_h[:], wsb[:])
        # fp8 copy of the small-|W| chunk for the fp8 matmul (the host folds
        # no scaling: values are well inside e4m3 range)
        wsb_8 = const_pool.tile([128, 1], F8)
        nc.vector.tensor_copy(wsb_8[:], wsb[:, 1:2])

        y_r = y_in.ap().rearrange("(t p u) -> t p u", p=128, u=G)
        y_tiles = {}

        def load_y(T):
            y_t = spool.tile([128, G], F32, tag="y")
            nc.sync.dma_start(y_t[:], y_r[T])
            y_tiles[T] = y_t

        ett_tiles = {}

        def load_tile(T, split=False):
            ett16 = epool.tile([128, RPT], BF16, tag="ett16")
            ett8 = epool.tile([128, RPT], F8, tag="ett8")
            if split:
                # half-col extents so the first chunks' matmuls start
                # earlier (pipeline head only)
                h = RPT // 2
                for lo, hi in ((0, h), (h, RPT)):
                    nc.sync.dma_start(ett16[:, lo:hi],
                                      et16_in.ap()[T][:, lo:hi])
                    nc.sync.dma_start(ett8[:, lo:hi],
                                      et8_in.ap()[T][:, lo:hi])
            else:
                nc.sync.dma_start(ett16[:], et16_in.ap()[T])
                nc.sync.dma_start(ett8[:], et8_in.ap()[T])
            ett_tiles[T] = (ett16, ett8)

        # first y tiles ahead of the encode stream (128 KiB, ~0.4 us)
        for T in range(4):
            load_y(T)
        load_tile(0, split=True)
        load_tile(1)

        # free-dim iota (elem index within group) for the sort keys
        iota_i = const_pool.tile([128, G], I32)
        nc.gpsimd.iota(iota_i[:], pattern=[[1, G]], base=0, channel_multiplier=0)
        # descending iota (63..0) as int16: data for the rank-producing scatter
        iota_d16 = const_pool.tile([128, G], I16)
        nc.gpsimd.iota(iota_d16[:], pattern=[[-1, G]], base=G - 1,
                       channel_multiplier=0)

        sflat_tiles = {}
        s_tiles = {}
        rank_tiles = {}
        ss_tiles = {}
        th_tiles = {}

        def rank_chain(T):
            """Keys + sort + rank for tile T (deps: y tile loaded 4 tiles
            ahead + iotas; never waits at execution time)."""
            y_t = y_tiles.pop(T)
            # keys on DVE: k64 = int(y*2^29 + 2^29) (exact in fp32: y is a
            # multiple of 2^-23), keys = k64 | elem_index
            k64 = spool.tile([128, G], I32, tag="k64")
            nc.vector.tensor_scalar(out=k64[:], in0=y_t[:],
                                    scalar1=float(1 << 29),
                                    scalar2=float(1 << 29),
                                    op0=Alu.mult, op1=Alu.add)
            keys = spool.tile([128, G], I32, tag="keys")
            nc.vector.tensor_tensor(out=keys[:], in0=k64[:], in1=iota_i[:],
                                    op=Alu.bitwise_or)

            # full descending sort of the int keys on DVE (f32 bitcast views
            # order identically to the positive int32 keys)
            sorted_i = spool.tile([128, G], I32, tag="sorted")
            wka = scr_pool.tile([128, G], I32, tag="wka")
            wkb = scr_pool.tile([128, G], I32, tag="wkb")
            src = keys
            dst_t = wka
            for r in range(8):
                nc.vector.max(sorted_i[:, r * 8:(r + 1) * 8].bitcast(F32),
                              src[:].bitcast(F32))
                if r < 7:
                    nc.vector.match_replace(
                        dst_t[:].bitcast(F32),
                        sorted_i[:, r * 8:(r + 1) * 8].bitcast(F32),
                        src[:].bitcast(F32), 0.0)
                    src, dst_t = dst_t, (wkb if dst_t is wka else wka)

            # perm (descending argsort) = low 6 bits; int16 for the scatter
            perm32 = scr_pool.tile([128, G], I32, tag="perm32")
            nc.vector.tensor_scalar(out=perm32[:], in0=sorted_i[:],
                                    scalar1=63, scalar2=None,
                                    op0=Alu.bitwise_and)
            perm16 = spool.tile([128, G], I16, tag="perm16")
            nc.vector.tensor_copy(perm16[:], perm32[:])
            # rank_asc[i] = position of element i in ascending order
            rank16 = spool.tile([128, G], I16, tag="rank16")
            nc.gpsimd.local_scatter(rank16[:], iota_d16[:], perm16[:],
                                    channels=128, num_elems=G, num_idxs=G)
            rank_tiles[T] = rank16

        def score_scatter(T):
            s_t = s_tiles.pop(T)
            rank16 = rank_tiles.pop(T)
            ssort = spool.tile([128, G], I16, tag="ssort")
            nc.gpsimd.local_scatter(ssort[:], s_t[:].bitcast(I16), rank16[:],
                                    channels=128, num_elems=G, num_idxs=G)
            ss_tiles[T] = ssort

        def tanh_store(T):
            ssort = ss_tiles.pop(T)
            ssf = ssort[:].bitcast(F16)
            negs0 = spool.tile([128, 1], F32, tag="negs0")
            nc.scalar.mul(negs0[:], ssf[:, 0:1], -1.0)
            th = spool.tile([128, G - 1], F32, tag="th")
            nc.scalar.activation(th[:], ssf[:, 1:G], Act.Tanh,
                                 bias=negs0[:], scale=1.0)
            th_tiles[T] = th

        def store_out(T):
            nc.sync.dma_start(out_r[T], th_tiles.pop(T))

        def relayout(T):
            s_t = spool.tile([128, G], F16, tag="s")
            nc.sync.dma_start(s_t[:], sflat_tiles.pop(T)[:])
            s_tiles[T] = s_t

        # rank chains for the first two tiles run during the DMA head
        rank_chain(0)
        rank_chain(1)

        for T in range(T_TILES):
            if T + 2 < T_TILES:
                load_tile(T + 2)
            ett16, ett8 = ett_tiles.pop(T)

            # ---- scores for tile T (native-orientation PE matvec) ----
            sflat = sfpool.tile([1, RPT], F16, tag="sflat")
            for j in range(RPT // EXIT_N):
                ps = ps_pool.tile([1, EXIT_N], F32, tag="ps")
                for q in range(EXIT_N // MM_N):
                    c0 = j * EXIT_N + q * MM_N
                    nc.tensor.matmul(ps[:, q * MM_N:(q + 1) * MM_N],
                                     wsb_h[:, 0:1], ett16[:, c0:c0 + MM_N],
                                     start=True, stop=False)
                    nc.tensor.matmul(ps[:, q * MM_N:(q + 1) * MM_N],
                                     wsb_8[:], ett8[:, c0:c0 + MM_N],
                                     start=False, stop=True)
                # split exit: Act + DVE halves run concurrently
                base = j * EXIT_N
                nc.scalar.copy(sflat[:, base:base + EXIT_SPLIT],
                               ps[:, 0:EXIT_SPLIT])
                nc.vector.tensor_copy(sflat[:, base + EXIT_SPLIT:base + EXIT_N],
                                      ps[:, EXIT_SPLIT:EXIT_N])

            sflat_tiles[T] = sflat

            # lagged, decoupled tail stages.  relayout at lag 2: at lag 1
            # its dep (the last exit of T-1) completes a hair after emission
            # and the wait at the sync-ring head delays every later encode
            # transfer by ~1.5 us/tile.
            if T >= 2:
                relayout(T - 2)
            if T + 4 < T_TILES:
                load_y(T + 4)
            if T + 2 < T_TILES:
                rank_chain(T + 2)
            if T >= 3:
                score_scatter(T - 3)
            if T >= 4:
                tanh_store(T - 4)
            if T >= 5:
                store_out(T - 5)

        TL = T_TILES - 1
        for T in (TL - 1, TL):
            relayout(T)
        for T in (TL - 2, TL - 1, TL):
            score_scatter(T)
        for T in (TL - 3, TL - 2, TL - 1, TL):
            tanh_store(T)
        for T in (TL - 4, TL - 3, TL - 2, TL - 1, TL):
            store_out(T)

    nc.compile()
    return nc


last_results = None


def kernel(encodes, y_coord, W, x_coord=None):
    global last_results
    if "nc" not in _built:
        _built["nc"] = _build_nc()
    nc = _built["nc"]

    encodes = np.asarray(encodes, dtype=np.float32)
    y_coord = np.ascontiguousarray(np.asarray(y_coord, dtype=np.float32))
    w_flat = np.asarray(W, dtype=np.float32).ravel()
    # permute dims by descending |W| (z = sum_d W[d]E[d] is invariant):
    # big-|W| half streams as bf16, small-|W| half as fp8-e4m3
    order = np.argsort(-np.abs(w_flat))
    w_perm = np.ascontiguousarray(w_flat[order].reshape(1, D))

    in_maps = []
    for c in range(N_CORES):
        e_c = encodes[c * ROWS:(c + 1) * ROWS][:, order]
        big = e_c[:, :128].astype(ml_dtypes.bfloat16)
        small = e_c[:, 128:].astype(ml_dtypes.float8_e4m3fn)
        # [ROWS, 128] -> [128, ROWS] -> [128, 16, RPT] -> [16, 128, RPT]:
        # each [T] slice one dense linear extent, partition (d) contiguous
        et16_c = np.ascontiguousarray(
            big.T.reshape(128, T_TILES, RPT).transpose(1, 0, 2))
        et8_c = np.ascontiguousarray(
            small.T.reshape(128, T_TILES, RPT).transpose(1, 0, 2))
        in_maps.append({
            "et16": et16_c,
            "et8": et8_c,
            "y_coord": y_coord[c * ROWS:(c + 1) * ROWS],
            "w": w_perm,
        })
    # Only request tracing when the axon NTFF hook is importable; otherwise
    # force it off (bass_utils would crash importing antenv.axon_hooks if
    # BASS_TRACE leaked into the environment without the shim installed).
    want_trace = bool(os.environ.get("BASS_TRACE"))
    if want_trace:
        try:
            from antenv.axon_hooks import get_axon_ntff_profile_hook  # noqa: F401
        except ImportError:
            want_trace = False
            os.environ["BASS_NEVER_TRACE"] = "1"
    res = run_bass_kernel_spmd(
        nc, in_maps, core_ids=list(range(N_CORES)),
        trace=want_trace,
    )
    last_results = res
    result = np.concatenate([r["result"] for r in res.results])
    polarity = np.ones(NG * (G - 1), dtype=np.float32)
    return result, polarity
